# revision 26
# baseline (speedup 1.0000x reference)
"""AssignAttention (hard-routing slot attention) Trainium2 kernel, 8-core data-parallel.

Problem: B=16, N=64 groups, S=4096 tokens, C=768, H=8 heads, HD=96.
  q = query @ Wq.T; k = key @ Wk.T; v = key @ Wv.T (per-head split)
  logits = q @ k.T (argmax over the 64 groups per token -> one-hot; softmax and
  the *SCALE factor are argmax-invariant so both are skipped entirely)
  attn = onehot / (count + 1); out = (attn @ v per head) @ Wo.T + bo

Sharding: data-parallel over batch B: 16 batches / 8 cores = 2 per core.
No collectives; host concatenates per-core outputs.

Per-core precision strategy (validated vs the fp32 reference: rel_l2 ~1e-3):
  - Q/K projections: fp16 hi/lo split, 3 matmuls (error ~2^-21, keeps argmax
    flips at the level caused by fp32 reassociation anyway)
  - QK^T logits: fp32 matmuls (contraction = 96)
  - V projection, group-sum, Wo projection: single-pass fp16 (linear-path error
    ~5e-4, well under the 2e-2 gate); bias added in fp32
  - All PSUM accumulation fp32

Layouts (per batch b):
  keyT (c-major) from fp16 hi/lo split of natural key rows + PE transpose-mode
  (fp16, 1 cyc/row), 6 blocks batched per PSUM bank, one strided copy out.
  kT [d, s] (d-major, 6x128 tiles) for logits lhsT; v [s, d] natural for group
  rhs. Group sums accumulate in PSUM [64, 8*128] across all of S (counts via
  N=1 ones-column matmul); one accumulation group per 2KB PSUM bank (start
  zeroes the whole bank). Per-partition reciprocal divide; attn_out [n, c]
  fp16; PE-transpose to attn_outT; Wo matmul; bias via K=1 outer-product.
"""

import sys

if "/opt/trn_rl_repo" not in sys.path:
    sys.path.insert(0, "/opt/trn_rl_repo")

import numpy as np

import concourse.bass as bass
import concourse.mybir as mybir
from concourse import bacc
import concourse.tile as tile
from concourse.masks import make_identity

f32 = mybir.dt.float32
f16 = mybir.dt.float16

C = 768
H = 8
HD = 96
NG = 64  # groups
CT = C // 128  # 6 c-tiles
S_CHUNK = 512


def _head_ranges(h):
    """Decompose head h's c-range [96h, 96h+96) into (tile, offset, size) pieces
    satisfying the matmul tile_position rules: size<=32 -> offset in {0,32,64,96},
    size<=64 -> offset in {0,64}, else offset 0."""
    lo, hi = HD * h, HD * h + HD
    out = []
    pos = lo
    while pos < hi:
        t, off = divmod(pos, 128)
        size = min(hi, (t + 1) * 128) - pos
        if off == 0:
            take = size
        elif off == 32:
            take = min(size, 32)
        elif off == 64:
            take = min(size, 64)
        elif off == 96:
            take = min(size, 32)
        else:
            raise AssertionError(off)
        out.append((t, off, take))
        pos += take
    return out


def build_nc(b_sh=2, S=4096):
    nc = bacc.Bacc()

    query_d = nc.declare_dram_parameter("query", [b_sh, NG, C], f32, isOutput=False)
    key_d = nc.declare_dram_parameter("key_in", [b_sh, S, C], f32, isOutput=False)
    wq_d = nc.declare_dram_parameter("Wq", [C, C], f32, isOutput=False)
    wk_d = nc.declare_dram_parameter("Wk", [C, C], f32, isOutput=False)
    wv_d = nc.declare_dram_parameter("Wv", [C, C], f32, isOutput=False)
    wo_d = nc.declare_dram_parameter("Wo", [C, C], f32, isOutput=False)
    bo_d = nc.declare_dram_parameter("bo", [C], f32, isOutput=False)
    out_d = nc.declare_dram_parameter("out", [b_sh, NG, C], f32, isOutput=True)

    n_chunks = S // S_CHUNK
    n_sub = S_CHUNK // 128  # s-subtiles per chunk

    with tile.TileContext(nc) as tc:
        with (
            tc.tile_pool(name="wconst", bufs=1) as wconst,
            tc.tile_pool(name="qpool", bufs=1) as qpool,
            tc.tile_pool(name="kin", bufs=2) as kin,
            tc.tile_pool(name="keyT", bufs=2) as keyTp,
            tc.tile_pool(name="vp", bufs=3) as vp,
            tc.tile_pool(name="ohp", bufs=3) as ohp,
            tc.tile_pool(name="mxp", bufs=3) as mxp,
            tc.tile_pool(name="outp", bufs=1) as outp,
            tc.tile_pool(name="ps_a", bufs=2, space="PSUM") as ps_a,
            tc.tile_pool(name="ps_v", bufs=2, space="PSUM") as ps_v,
            tc.tile_pool(name="ps_tr", bufs=2, space="PSUM") as ps_tr,
            tc.tile_pool(name="ps_gs", bufs=1, space="PSUM") as ps_gs,
        ):
            # ---- constants ----
            ident64_16 = wconst.tile([NG, NG], f16)
            make_identity(nc, ident64_16[:])
            ident64_32 = wconst.tile([NG, NG], f32)
            make_identity(nc, ident64_32[:])
            ident128_16 = wconst.tile([128, 128], f16)
            make_identity(nc, ident128_16[:])
            ident128_32 = wconst.tile([128, 128], f32)
            make_identity(nc, ident128_32[:])
            ones_col = wconst.tile([128, 1], f16)
            nc.vector.memset(ones_col[:], 1.0)
            ones_row = wconst.tile([1, NG], f32)
            nc.vector.memset(ones_row[:], 1.0)
            bo_sb = wconst.tile([1, C], f32)
            nc.sync.dma_start(out=bo_sb[:], in_=bo_d[:].unsqueeze(0))

            def pe_transpose_blocks_f32(src, dst, t):
                """PE-transpose 6 f32 [128,128] blocks src[:, 128u:128u+128]
                into dst[:, u, 128t:128t+128]; 3 blocks per PSUM bank."""
                for g in range(2):
                    trp = ps_tr.tile([128, 3, 128], f32, tag="pstr")
                    for j in range(3):
                        u = 3 * g + j
                        nc.tensor.matmul(
                            trp[:, j, :],
                            src[:, 128 * u : 128 * u + 128],
                            ident128_32[:],
                            is_transpose=True,
                            start=(j == 0),
                            stop=(j == 2),
                        )
                    nc.scalar.copy(
                        out=dst[:, 3 * g : 3 * g + 3, 128 * t : 128 * t + 128],
                        in_=trp[:],
                    )

            def pe_transpose_blocks(src, dst, t, rows=128):
                """PE-transpose 6 f16 [rows,128] blocks src[:, 128u:128u+128]
                (u=0..5) into dst[:, u, 128t:128t+rows] via one batched PSUM
                bank + a single strided ACT copy."""
                ident = ident128_16 if rows == 128 else ident64_16
                trp = ps_tr.tile([128, CT, rows], f16, tag="pstr")
                for u in range(CT):
                    nc.tensor.matmul(
                        trp[:, u, :],
                        src[0:rows, 128 * u : 128 * u + 128],
                        ident[:],
                        is_transpose=True,
                        start=(u == 0),
                        stop=(u == CT - 1),
                    )
                nc.scalar.copy(out=dst[:, :, 128 * t : 128 * t + rows], in_=trp[:])

            # ---- weight prep: transpose to c-major fp16 hi/lo ----
            # wT[p, u, d] = W[d, 128u + p]
            CP = 128 * H  # d-padded width for Wq/Wk (head h at 128h..128h+96)
            wqT_h = wconst.tile([128, CT, CP], f16)
            wqT_l = wconst.tile([128, CT, CP], f16)
            # Wk kept NATURAL fp32 (d-padded rows, c free) for the
            # Y = Wk_h^T q_h precompute; logits = keyT^T @ Y in fp32
            # (~2.1 cyc/row warm measured, and exact precision)
            wk_nat = wconst.tile([128, H, C], f32)
            wvT_h = wconst.tile([128, CT, C], f16)
            woT_h = wconst.tile([128, CT, C], f16)

            # Wq/Wk use the d-padded layout (head h -> cols 128h..128h+96, rest
            # zero) so every logits matmul is a single offset-0 K=128 matmul
            # (mixed-row-offset PSUM accumulation groups fail to load on HW).
            wtmp_ctx = tc.tile_pool(name="wtmp", bufs=2)
            wtmp = wtmp_ctx.__enter__()
            for hd in range(H):
                wnat = wtmp.tile([128, C], f32, tag="wnat")
                nc.vector.memset(wnat[96:128, :], 0.0)
                nc.sync.dma_start(
                    out=wnat[0:HD, :], in_=wq_d[HD * hd : HD * hd + HD, :]
                )
                whi = wtmp.tile([128, C], f16, tag="whi")
                nc.vector.tensor_copy(whi[:], wnat[:])
                pe_transpose_blocks(whi[:], wqT_h[:], hd)
                wlo = wtmp.tile([128, C], f16, tag="wlo")
                nc.vector.tensor_tensor(
                    out=wlo[:], in0=wnat[:], in1=whi[:], op=mybir.AluOpType.subtract
                )
                pe_transpose_blocks(wlo[:], wqT_l[:], hd)
            nc.vector.memset(wk_nat[96:128, :, :], 0.0)
            for hd in range(H):
                nc.sync.dma_start(
                    out=wk_nat[0:HD, hd, :], in_=wk_d[HD * hd : HD * hd + HD, :]
                )
            for w_dram, dst_h in ((wv_d, wvT_h), (wo_d, woT_h)):
                for t in range(CT):
                    wnat = wtmp.tile([128, C], f32, tag="wnat")
                    nc.sync.dma_start(out=wnat[:], in_=w_dram[128 * t : 128 * t + 128, :])
                    whi = wtmp.tile([128, C], f16, tag="whi")
                    nc.vector.tensor_copy(whi[:], wnat[:])
                    pe_transpose_blocks(whi[:], dst_h[:], t)
            wtmp_ctx.__exit__(None, None, None)

            for b in range(b_sh):
                # ---- Q path ----
                q_nat = qpool.tile([NG, C], f32, tag="qnat")
                nc.sync.dma_start(out=q_nat[:], in_=query_d[b])
                qh_nat = qpool.tile([NG, C], f16, tag="qhnat")
                ql_nat = qpool.tile([NG, C], f16, tag="qlnat")
                nc.vector.tensor_copy(qh_nat[:], q_nat[:])
                nc.vector.tensor_tensor(
                    out=ql_nat[:], in0=q_nat[:], in1=qh_nat[:], op=mybir.AluOpType.subtract
                )
                # queryT (c-major) fp16 halves via DMA xbar ([64,128] blocks)
                qTq_h = qpool.tile([128, CT, NG], f16, tag="qTqh")
                qTq_l = qpool.tile([128, CT, NG], f16, tag="qTql")
                for qsrc, dst in ((qh_nat, qTq_h), (ql_nat, qTq_l)):
                    pe_transpose_blocks(qsrc, dst[:].unsqueeze(3).rearrange("p u n o -> p u (n o)"), 0, rows=NG)
                # q projection (natural layout, M=64), d-padded: q_pad [64, 1024]
                q_sb = qpool.tile([NG, CP], f32, tag="qsb")
                for half in range(2):
                    nsl = slice(512 * half, 512 * half + 512)
                    qp = ps_a.tile([NG, 512], f32, tag="psa")
                    first = True
                    for u in range(CT):
                        for lhsT, rhs in (
                            (qTq_h, wqT_h),
                            (qTq_h, wqT_l),
                            (qTq_l, wqT_h),
                        ):
                            nc.tensor.matmul(
                                qp[:],
                                lhsT[:, u, :],
                                rhs[:, u, nsl],
                                start=first,
                                stop=(u == CT - 1 and lhsT is qTq_l),
                            )
                            first = False
                    nc.scalar.copy(out=q_sb[:, nsl], in_=qp[:])
                # qT (padded d-major, per head) fp32 via PE transpose
                qT = qpool.tile([128, H, NG], f32, tag="qT")
                for hd in range(H):
                    trq2 = ps_a.tile([128, NG], f32, tag="psa")
                    nc.tensor.matmul(
                        trq2[:],
                        q_sb[:, 128 * hd : 128 * hd + 128],
                        ident64_32[:],
                        is_transpose=True,
                        start=True,
                        stop=True,
                    )
                    nc.scalar.copy(out=qT[:, hd, :], in_=trq2[:])
                # Y_all[c, 64h+n] = sum_d Wk[d(head h), c] * q[n, d], fp32,
                # then fp16 hi/lo split. logits = keyT^T @ Y_all (split x3).
                Y_h = qpool.tile([128, CT, 8 * NG], f16, tag="Yh")
                Y_l = qpool.tile([128, CT, 8 * NG], f16, tag="Yl")
                for u_c in range(CT):
                    yp = ps_a.tile([128, 8 * NG], f32, tag="psa")
                    csl = slice(128 * u_c, 128 * u_c + 128)
                    for hd in range(H):
                        nc.tensor.matmul(
                            yp[:, NG * hd : NG * hd + NG],
                            wk_nat[:, hd, csl],
                            qT[:, hd, :],
                            start=(hd == 0),
                            stop=(hd == H - 1),
                        )
                    nc.vector.tensor_copy(Y_h[:, u_c, :], yp[:])
                    nc.vector.tensor_tensor(
                        out=Y_l[:, u_c, :], in0=yp[:], in1=Y_h[:, u_c, :],
                        op=mybir.AluOpType.subtract,
                    )

                # ---- group-sum accumulator for this b ----
                gs = ps_gs.tile([NG, 8 * 128], f32, tag="gs")

                for chunk in range(n_chunks):
                    s0 = chunk * S_CHUNK
                    # load + split key chunk
                    knat = kin.tile([128, n_sub, C], f32, tag="knat")
                    nc.gpsimd.dma_start(
                        out=knat[:],
                        in_=key_d[b, s0 : s0 + S_CHUNK, :].rearrange(
                            "(i p) c -> p i c", p=128
                        ),
                    )
                    # keyT via fp32 PE transposes, then fp16 hi/lo split
                    # (hi also feeds the v-projection)
                    keyT = keyTp.tile([128, CT, S_CHUNK], f32, tag="keyT")
                    for i in range(n_sub):
                        pe_transpose_blocks_f32(knat[:, i, :], keyT[:], i)
                    kTh = keyTp.tile([128, CT, S_CHUNK], f16, tag="kTh")
                    kTl = keyTp.tile([128, CT, S_CHUNK], f16, tag="kTl")
                    nc.vector.tensor_copy(kTh[:], keyT[:])
                    nc.vector.tensor_tensor(
                        out=kTl[:], in0=keyT[:], in1=kTh[:], op=mybir.AluOpType.subtract
                    )

                    for i in range(n_sub):
                        ssl = slice(128 * i, 128 * i + 128)
                        # v projection (natural [s, d]) fp16 single pass;
                        # two single-bank PSUM tiles so copies pipeline
                        vpsA = ps_v.tile([128, 384], f32, tag="vps")
                        vpsB = ps_v.tile([128, 384], f32, tag="vps")
                        for u_c in range(CT):
                            nc.tensor.matmul(
                                vpsA[:],
                                kTh[:, u_c, ssl],
                                wvT_h[:, u_c, 0:384],
                                start=(u_c == 0),
                                stop=(u_c == CT - 1),
                            )
                            nc.tensor.matmul(
                                vpsB[:],
                                kTh[:, u_c, ssl],
                                wvT_h[:, u_c, 384:768],
                                start=(u_c == 0),
                                stop=(u_c == CT - 1),
                            )
                        v16 = vp.tile([128, C], f16, tag="v16")
                        nc.scalar.copy(out=v16[:, 0:384], in_=vpsA[:])
                        nc.scalar.copy(out=v16[:, 384:768], in_=vpsB[:])

                        # logits for all 8 heads at once: lg[s, 64h+n] =
                        # sum_c keyT[c, s] Y_all[c, 64h+n], fp16 split x3.
                        # One accumulation group per PSUM bank: start only on
                        # the first matmul (zeroes the 2KB region), stop last.
                        lg = ps_a.tile([128, 8 * NG], f32, tag="psa")
                        first = True
                        for u_c in range(CT):
                            for kt, yt in ((kTh, Y_h), (kTh, Y_l), (kTl, Y_h)):
                                nc.tensor.matmul(
                                    lg[:],
                                    kt[:, u_c, ssl],
                                    yt[:, u_c, :],
                                    start=first,
                                    stop=(u_c == CT - 1 and kt is kTl),
                                )
                                first = False
                        # argmax -> one-hot via (x >= rowmax), fp16
                        mx = mxp.tile([128, H], f32, tag="mx")
                        lg3 = lg[:].rearrange("p (h n) -> p h n", h=H)
                        nc.vector.tensor_reduce(
                            out=mx[:],
                            in_=lg3,
                            axis=mybir.AxisListType.X,
                            op=mybir.AluOpType.max,
                        )
                        oh = ohp.tile([128, H * NG], f16, tag="oh")
                        nc.vector.tensor_tensor(
                            out=oh[:].rearrange("p (h n) -> p h n", h=H),
                            in0=lg3,
                            in1=mx[:].unsqueeze(2).to_broadcast((128, H, NG)),
                            op=mybir.AluOpType.is_ge,
                        )

                        # group sums + counts (fp16 matmuls, fp32 accum)
                        # gs spans 2 PSUM banks (heads 0-3, heads 4-7): one
                        # start per bank (zeroes the 2KB region), one stop
                        last = chunk == n_chunks - 1 and i == n_sub - 1
                        first = chunk == 0 and i == 0
                        for h in range(H):
                            lh = oh[:, NG * h : NG * h + NG]
                            nc.tensor.matmul(
                                gs[:, 128 * h : 128 * h + HD],
                                lh,
                                v16[:, HD * h : HD * h + HD],
                                start=(first and h in (0, 4)),
                                stop=False,
                            )
                            nc.tensor.matmul(
                                gs[:, 128 * h + HD : 128 * h + HD + 1],
                                lh,
                                ones_col[:],
                                start=False,
                                stop=(last and h in (3, 7)),
                            )

                # ---- finalize b: divide by (count+1), transpose, Wo, bias ----
                cnt = outp.tile([NG, H], f32, tag="cnt")
                nc.vector.tensor_scalar(
                    out=cnt[:],
                    in0=gs[:].rearrange("p (h q) -> p h q", q=128)[:, :, HD],
                    scalar1=1.0,
                    scalar2=None,
                    op0=mybir.AluOpType.add,
                )
                rec = outp.tile([NG, H], f32, tag="rec")
                nc.vector.reciprocal(rec[:], cnt[:])
                attn16 = outp.tile([NG, C], f16, tag="attn16")
                for h in range(H):
                    nc.vector.tensor_scalar(
                        out=attn16[:, HD * h : HD * h + HD],
                        in0=gs[:, 128 * h : 128 * h + HD],
                        scalar1=rec[:, h : h + 1],
                        scalar2=None,
                        op0=mybir.AluOpType.mult,
                    )
                attnT = outp.tile([128, CT, NG], f16, tag="attnT")
                pe_transpose_blocks(attn16, attnT[:].unsqueeze(3).rearrange("p u n o -> p u (n o)"), 0, rows=NG)

                out_sb = outp.tile([NG, C], f32, tag="outsb")
                for half in range(2):
                    nsl = slice(384 * half, 384 * half + 384)
                    op = ps_a.tile([NG, 384], f32, tag="psa")
                    for u_c in range(CT):
                        nc.tensor.matmul(
                            op[:],
                            attnT[:, u_c, :],
                            woT_h[:, u_c, nsl],
                            start=(u_c == 0),
                            stop=False,
                        )
                    nc.tensor.matmul(
                        op[:], ones_row[:], bo_sb[:, nsl], start=False, stop=True
                    )
                    nc.scalar.copy(out=out_sb[:, nsl], in_=op[:])
                nc.gpsimd.dma_start(out=out_d[b], in_=out_sb[:])

    nc.finalize()
    return nc


_NC_CACHE = {}


def _get_nc(b_sh, S):
    key = (b_sh, S)
    if key not in _NC_CACHE:
        _NC_CACHE[key] = build_nc(b_sh, S)
    return _NC_CACHE[key]


def kernel(query, key_in, Wq, Wk, Wv, Wo, bo):
    from concourse.bass_utils import run_bass_kernel_spmd

    query = np.ascontiguousarray(np.asarray(query, dtype=np.float32))
    key_in = np.ascontiguousarray(np.asarray(key_in, dtype=np.float32))
    Wq = np.ascontiguousarray(np.asarray(Wq, dtype=np.float32))
    Wk = np.ascontiguousarray(np.asarray(Wk, dtype=np.float32))
    Wv = np.ascontiguousarray(np.asarray(Wv, dtype=np.float32))
    Wo = np.ascontiguousarray(np.asarray(Wo, dtype=np.float32))
    bo = np.ascontiguousarray(np.asarray(bo, dtype=np.float32))

    B, _, _ = query.shape
    S = key_in.shape[1]
    n_cores = 8
    b_sh = B // n_cores
    nc = _get_nc(b_sh, S)

    in_maps = []
    for i in range(n_cores):
        bs = slice(i * b_sh, (i + 1) * b_sh)
        in_maps.append(
            {
                "query": np.ascontiguousarray(query[bs]),
                "key_in": np.ascontiguousarray(key_in[bs]),
                "Wq": Wq,
                "Wk": Wk,
                "Wv": Wv,
                "Wo": Wo,
                "bo": bo,
            }
        )
    res = run_bass_kernel_spmd(nc, in_maps, core_ids=list(range(n_cores)))
    out = np.concatenate([res.results[i]["out"] for i in range(n_cores)], axis=0)
    return out.astype(np.float32)


if __name__ == "__main__":
    nc = build_nc(1, 512)
    print("built ok")


# revision 27
# speedup vs baseline: 1.0107x; 1.0107x over previous
"""AssignAttention (hard-routing slot attention) Trainium2 kernel, 8-core data-parallel.

Problem: B=16, N=64 groups, S=4096 tokens, C=768, H=8 heads, HD=96.
  q = query @ Wq.T; k = key @ Wk.T; v = key @ Wv.T (per-head split)
  logits = q @ k.T (argmax over the 64 groups per token -> one-hot; softmax and
  the *SCALE factor are argmax-invariant so both are skipped entirely)
  attn = onehot / (count + 1); out = (attn @ v per head) @ Wo.T + bo

Sharding: data-parallel over batch B: 16 batches / 8 cores = 2 per core.
No collectives; host concatenates per-core outputs.

Per-core precision strategy (validated vs the fp32 reference: rel_l2 ~1e-3):
  - Q/K projections: fp16 hi/lo split, 3 matmuls (error ~2^-21, keeps argmax
    flips at the level caused by fp32 reassociation anyway)
  - QK^T logits: fp32 matmuls (contraction = 96)
  - V projection, group-sum, Wo projection: single-pass fp16 (linear-path error
    ~5e-4, well under the 2e-2 gate); bias added in fp32
  - All PSUM accumulation fp32

Layouts (per batch b):
  keyT (c-major) from fp16 hi/lo split of natural key rows + PE transpose-mode
  (fp16, 1 cyc/row), 6 blocks batched per PSUM bank, one strided copy out.
  kT [d, s] (d-major, 6x128 tiles) for logits lhsT; v [s, d] natural for group
  rhs. Group sums accumulate in PSUM [64, 8*128] across all of S (counts via
  N=1 ones-column matmul); one accumulation group per 2KB PSUM bank (start
  zeroes the whole bank). Per-partition reciprocal divide; attn_out [n, c]
  fp16; PE-transpose to attn_outT; Wo matmul; bias via K=1 outer-product.
"""

import sys

if "/opt/trn_rl_repo" not in sys.path:
    sys.path.insert(0, "/opt/trn_rl_repo")

import numpy as np

import concourse.bass as bass
import concourse.mybir as mybir
from concourse import bacc
import concourse.tile as tile
from concourse.masks import make_identity

f32 = mybir.dt.float32
f16 = mybir.dt.float16

C = 768
H = 8
HD = 96
NG = 64  # groups
CT = C // 128  # 6 c-tiles
S_CHUNK = 256


def _head_ranges(h):
    """Decompose head h's c-range [96h, 96h+96) into (tile, offset, size) pieces
    satisfying the matmul tile_position rules: size<=32 -> offset in {0,32,64,96},
    size<=64 -> offset in {0,64}, else offset 0."""
    lo, hi = HD * h, HD * h + HD
    out = []
    pos = lo
    while pos < hi:
        t, off = divmod(pos, 128)
        size = min(hi, (t + 1) * 128) - pos
        if off == 0:
            take = size
        elif off == 32:
            take = min(size, 32)
        elif off == 64:
            take = min(size, 64)
        elif off == 96:
            take = min(size, 32)
        else:
            raise AssertionError(off)
        out.append((t, off, take))
        pos += take
    return out


def build_nc(b_sh=2, S=4096):
    nc = bacc.Bacc()

    query_d = nc.declare_dram_parameter("query", [b_sh, NG, C], f32, isOutput=False)
    key_d = nc.declare_dram_parameter("key_in", [b_sh, S, C], f32, isOutput=False)
    wq_d = nc.declare_dram_parameter("Wq", [C, C], f32, isOutput=False)
    wk_d = nc.declare_dram_parameter("Wk", [C, C], f32, isOutput=False)
    wv_d = nc.declare_dram_parameter("Wv", [C, C], f32, isOutput=False)
    wo_d = nc.declare_dram_parameter("Wo", [C, C], f32, isOutput=False)
    bo_d = nc.declare_dram_parameter("bo", [C], f32, isOutput=False)
    out_d = nc.declare_dram_parameter("out", [b_sh, NG, C], f32, isOutput=True)

    n_chunks = S // S_CHUNK
    n_sub = S_CHUNK // 128  # s-subtiles per chunk

    with tile.TileContext(nc) as tc:
        with (
            tc.tile_pool(name="wconst", bufs=1) as wconst,
            tc.tile_pool(name="qpool", bufs=1) as qpool,
            tc.tile_pool(name="kin", bufs=2) as kin,
            tc.tile_pool(name="keyT", bufs=2) as keyTp,
            tc.tile_pool(name="vp", bufs=3) as vp,
            tc.tile_pool(name="ohp", bufs=3) as ohp,
            tc.tile_pool(name="mxp", bufs=3) as mxp,
            tc.tile_pool(name="outp", bufs=1) as outp,
            tc.tile_pool(name="ps_a", bufs=2, space="PSUM") as ps_a,
            tc.tile_pool(name="ps_v", bufs=2, space="PSUM") as ps_v,
            tc.tile_pool(name="ps_tr", bufs=2, space="PSUM") as ps_tr,
            tc.tile_pool(name="ps_gs", bufs=1, space="PSUM") as ps_gs,
        ):
            # ---- constants ----
            ident64_16 = wconst.tile([NG, NG], f16)
            make_identity(nc, ident64_16[:])
            ident64_32 = wconst.tile([NG, NG], f32)
            make_identity(nc, ident64_32[:])
            ident128_16 = wconst.tile([128, 128], f16)
            make_identity(nc, ident128_16[:])
            ident128_32 = wconst.tile([128, 128], f32)
            make_identity(nc, ident128_32[:])
            ones_col = wconst.tile([128, 1], f16)
            nc.vector.memset(ones_col[:], 1.0)
            ones_row = wconst.tile([1, NG], f32)
            nc.vector.memset(ones_row[:], 1.0)
            bo_sb = wconst.tile([1, C], f32)
            nc.sync.dma_start(out=bo_sb[:], in_=bo_d[:].unsqueeze(0))

            def pe_transpose_blocks_f32(src, dst, t):
                """PE-transpose 6 f32 [128,128] blocks src[:, 128u:128u+128]
                into dst[:, u, 128t:128t+128]; 3 blocks per PSUM bank."""
                for g in range(2):
                    trp = ps_tr.tile([128, 3, 128], f32, tag="pstr")
                    for j in range(3):
                        u = 3 * g + j
                        nc.tensor.matmul(
                            trp[:, j, :],
                            src[:, 128 * u : 128 * u + 128],
                            ident128_32[:],
                            is_transpose=True,
                            start=(j == 0),
                            stop=(j == 2),
                        )
                    nc.scalar.copy(
                        out=dst[:, 3 * g : 3 * g + 3, 128 * t : 128 * t + 128],
                        in_=trp[:],
                    )

            def pe_transpose_blocks(src, dst, t, rows=128):
                """PE-transpose 6 f16 [rows,128] blocks src[:, 128u:128u+128]
                (u=0..5) into dst[:, u, 128t:128t+rows] via one batched PSUM
                bank + a single strided ACT copy."""
                ident = ident128_16 if rows == 128 else ident64_16
                trp = ps_tr.tile([128, CT, rows], f16, tag="pstr")
                for u in range(CT):
                    nc.tensor.matmul(
                        trp[:, u, :],
                        src[0:rows, 128 * u : 128 * u + 128],
                        ident[:],
                        is_transpose=True,
                        start=(u == 0),
                        stop=(u == CT - 1),
                    )
                nc.scalar.copy(out=dst[:, :, 128 * t : 128 * t + rows], in_=trp[:])

            # ---- weight prep: transpose to c-major fp16 hi/lo ----
            # wT[p, u, d] = W[d, 128u + p]
            CP = 128 * H  # d-padded width for Wq/Wk (head h at 128h..128h+96)
            wqT_h = wconst.tile([128, CT, CP], f16)
            wqT_l = wconst.tile([128, CT, CP], f16)
            # Wk kept NATURAL fp32 (d-padded rows, c free) for the
            # Y = Wk_h^T q_h precompute; logits = keyT^T @ Y in fp32
            # (~2.1 cyc/row warm measured, and exact precision)
            wk_nat = wconst.tile([128, H, C], f32)
            wvT_h = wconst.tile([128, CT, C], f16)
            woT_h = wconst.tile([128, CT, C], f16)

            # Wq/Wk use the d-padded layout (head h -> cols 128h..128h+96, rest
            # zero) so every logits matmul is a single offset-0 K=128 matmul
            # (mixed-row-offset PSUM accumulation groups fail to load on HW).
            wtmp_ctx = tc.tile_pool(name="wtmp", bufs=2)
            wtmp = wtmp_ctx.__enter__()
            for hd in range(H):
                wnat = wtmp.tile([128, C], f32, tag="wnat")
                nc.vector.memset(wnat[96:128, :], 0.0)
                nc.sync.dma_start(
                    out=wnat[0:HD, :], in_=wq_d[HD * hd : HD * hd + HD, :]
                )
                whi = wtmp.tile([128, C], f16, tag="whi")
                nc.vector.tensor_copy(whi[:], wnat[:])
                pe_transpose_blocks(whi[:], wqT_h[:], hd)
                wlo = wtmp.tile([128, C], f16, tag="wlo")
                nc.vector.tensor_tensor(
                    out=wlo[:], in0=wnat[:], in1=whi[:], op=mybir.AluOpType.subtract
                )
                pe_transpose_blocks(wlo[:], wqT_l[:], hd)
            nc.vector.memset(wk_nat[96:128, :, :], 0.0)
            for hd in range(H):
                nc.sync.dma_start(
                    out=wk_nat[0:HD, hd, :], in_=wk_d[HD * hd : HD * hd + HD, :]
                )
            for w_dram, dst_h in ((wv_d, wvT_h), (wo_d, woT_h)):
                for t in range(CT):
                    wnat = wtmp.tile([128, C], f32, tag="wnat")
                    nc.sync.dma_start(out=wnat[:], in_=w_dram[128 * t : 128 * t + 128, :])
                    whi = wtmp.tile([128, C], f16, tag="whi")
                    nc.vector.tensor_copy(whi[:], wnat[:])
                    pe_transpose_blocks(whi[:], dst_h[:], t)
            wtmp_ctx.__exit__(None, None, None)

            for b in range(b_sh):
                # ---- Q path ----
                q_nat = qpool.tile([NG, C], f32, tag="qnat")
                nc.sync.dma_start(out=q_nat[:], in_=query_d[b])
                qh_nat = qpool.tile([NG, C], f16, tag="qhnat")
                ql_nat = qpool.tile([NG, C], f16, tag="qlnat")
                nc.vector.tensor_copy(qh_nat[:], q_nat[:])
                nc.vector.tensor_tensor(
                    out=ql_nat[:], in0=q_nat[:], in1=qh_nat[:], op=mybir.AluOpType.subtract
                )
                # queryT (c-major) fp16 halves via DMA xbar ([64,128] blocks)
                qTq_h = qpool.tile([128, CT, NG], f16, tag="qTqh")
                qTq_l = qpool.tile([128, CT, NG], f16, tag="qTql")
                for qsrc, dst in ((qh_nat, qTq_h), (ql_nat, qTq_l)):
                    pe_transpose_blocks(qsrc, dst[:].unsqueeze(3).rearrange("p u n o -> p u (n o)"), 0, rows=NG)
                # q projection (natural layout, M=64), d-padded: q_pad [64, 1024]
                q_sb = qpool.tile([NG, CP], f32, tag="qsb")
                for half in range(2):
                    nsl = slice(512 * half, 512 * half + 512)
                    qp = ps_a.tile([NG, 512], f32, tag="psa")
                    first = True
                    for u in range(CT):
                        for lhsT, rhs in (
                            (qTq_h, wqT_h),
                            (qTq_h, wqT_l),
                            (qTq_l, wqT_h),
                        ):
                            nc.tensor.matmul(
                                qp[:],
                                lhsT[:, u, :],
                                rhs[:, u, nsl],
                                start=first,
                                stop=(u == CT - 1 and lhsT is qTq_l),
                            )
                            first = False
                    nc.scalar.copy(out=q_sb[:, nsl], in_=qp[:])
                # qT (padded d-major, per head) fp32 via PE transpose
                qT = qpool.tile([128, H, NG], f32, tag="qT")
                for hd in range(H):
                    trq2 = ps_a.tile([128, NG], f32, tag="psa")
                    nc.tensor.matmul(
                        trq2[:],
                        q_sb[:, 128 * hd : 128 * hd + 128],
                        ident64_32[:],
                        is_transpose=True,
                        start=True,
                        stop=True,
                    )
                    nc.scalar.copy(out=qT[:, hd, :], in_=trq2[:])
                # Y_all[c, 64h+n] = sum_d Wk[d(head h), c] * q[n, d], fp32,
                # then fp16 hi/lo split. logits = keyT^T @ Y_all (split x3).
                Y_h = qpool.tile([128, CT, 8 * NG], f16, tag="Yh")
                Y_l = qpool.tile([128, CT, 8 * NG], f16, tag="Yl")
                for u_c in range(CT):
                    yp = ps_a.tile([128, 8 * NG], f32, tag="psa")
                    csl = slice(128 * u_c, 128 * u_c + 128)
                    for hd in range(H):
                        nc.tensor.matmul(
                            yp[:, NG * hd : NG * hd + NG],
                            wk_nat[:, hd, csl],
                            qT[:, hd, :],
                            start=(hd == 0),
                            stop=(hd == H - 1),
                        )
                    nc.vector.tensor_copy(Y_h[:, u_c, :], yp[:])
                    nc.vector.tensor_tensor(
                        out=Y_l[:, u_c, :], in0=yp[:], in1=Y_h[:, u_c, :],
                        op=mybir.AluOpType.subtract,
                    )

                # ---- group-sum accumulator for this b ----
                gs = ps_gs.tile([NG, 8 * 128], f32, tag="gs")

                for chunk in range(n_chunks):
                    s0 = chunk * S_CHUNK
                    # load + split key chunk
                    knat = kin.tile([128, n_sub, C], f32, tag="knat")
                    nc.gpsimd.dma_start(
                        out=knat[:],
                        in_=key_d[b, s0 : s0 + S_CHUNK, :].rearrange(
                            "(i p) c -> p i c", p=128
                        ),
                    )
                    # keyT via fp32 PE transposes, then fp16 hi/lo split
                    # (hi also feeds the v-projection)
                    keyT = keyTp.tile([128, CT, S_CHUNK], f32, tag="keyT")
                    for i in range(n_sub):
                        pe_transpose_blocks_f32(knat[:, i, :], keyT[:], i)
                    kTh = keyTp.tile([128, CT, S_CHUNK], f16, tag="kTh")
                    kTl = keyTp.tile([128, CT, S_CHUNK], f16, tag="kTl")
                    nc.vector.tensor_copy(kTh[:], keyT[:])
                    nc.vector.tensor_tensor(
                        out=kTl[:], in0=keyT[:], in1=kTh[:], op=mybir.AluOpType.subtract
                    )

                    for i in range(n_sub):
                        ssl = slice(128 * i, 128 * i + 128)
                        # v projection (natural [s, d]) fp16 single pass;
                        # two single-bank PSUM tiles so copies pipeline
                        vpsA = ps_v.tile([128, 384], f32, tag="vps")
                        vpsB = ps_v.tile([128, 384], f32, tag="vps")
                        for u_c in range(CT):
                            nc.tensor.matmul(
                                vpsA[:],
                                kTh[:, u_c, ssl],
                                wvT_h[:, u_c, 0:384],
                                start=(u_c == 0),
                                stop=(u_c == CT - 1),
                            )
                            nc.tensor.matmul(
                                vpsB[:],
                                kTh[:, u_c, ssl],
                                wvT_h[:, u_c, 384:768],
                                start=(u_c == 0),
                                stop=(u_c == CT - 1),
                            )
                        v16 = vp.tile([128, C], f16, tag="v16")
                        nc.scalar.copy(out=v16[:, 0:384], in_=vpsA[:])
                        nc.scalar.copy(out=v16[:, 384:768], in_=vpsB[:])

                        # logits for all 8 heads at once: lg[s, 64h+n] =
                        # sum_c keyT[c, s] Y_all[c, 64h+n], fp16 split x3.
                        # One accumulation group per PSUM bank: start only on
                        # the first matmul (zeroes the 2KB region), stop last.
                        lg = ps_a.tile([128, 8 * NG], f32, tag="psa")
                        first = True
                        for u_c in range(CT):
                            for kt, yt in ((kTh, Y_h), (kTh, Y_l), (kTl, Y_h)):
                                nc.tensor.matmul(
                                    lg[:],
                                    kt[:, u_c, ssl],
                                    yt[:, u_c, :],
                                    start=first,
                                    stop=(u_c == CT - 1 and kt is kTl),
                                )
                                first = False
                        # argmax -> one-hot via (x >= rowmax), fp16
                        mx = mxp.tile([128, H], f32, tag="mx")
                        lg3 = lg[:].rearrange("p (h n) -> p h n", h=H)
                        nc.vector.tensor_reduce(
                            out=mx[:],
                            in_=lg3,
                            axis=mybir.AxisListType.X,
                            op=mybir.AluOpType.max,
                        )
                        oh = ohp.tile([128, H * NG], f16, tag="oh")
                        nc.vector.tensor_tensor(
                            out=oh[:].rearrange("p (h n) -> p h n", h=H),
                            in0=lg3,
                            in1=mx[:].unsqueeze(2).to_broadcast((128, H, NG)),
                            op=mybir.AluOpType.is_ge,
                        )

                        # group sums + counts (fp16 matmuls, fp32 accum)
                        # gs spans 2 PSUM banks (heads 0-3, heads 4-7): one
                        # start per bank (zeroes the 2KB region), one stop
                        last = chunk == n_chunks - 1 and i == n_sub - 1
                        first = chunk == 0 and i == 0
                        for h in range(H):
                            lh = oh[:, NG * h : NG * h + NG]
                            nc.tensor.matmul(
                                gs[:, 128 * h : 128 * h + HD],
                                lh,
                                v16[:, HD * h : HD * h + HD],
                                start=(first and h in (0, 4)),
                                stop=False,
                            )
                            nc.tensor.matmul(
                                gs[:, 128 * h + HD : 128 * h + HD + 1],
                                lh,
                                ones_col[:],
                                start=False,
                                stop=(last and h in (3, 7)),
                            )

                # ---- finalize b: divide by (count+1), transpose, Wo, bias ----
                cnt = outp.tile([NG, H], f32, tag="cnt")
                nc.vector.tensor_scalar(
                    out=cnt[:],
                    in0=gs[:].rearrange("p (h q) -> p h q", q=128)[:, :, HD],
                    scalar1=1.0,
                    scalar2=None,
                    op0=mybir.AluOpType.add,
                )
                rec = outp.tile([NG, H], f32, tag="rec")
                nc.vector.reciprocal(rec[:], cnt[:])
                attn16 = outp.tile([NG, C], f16, tag="attn16")
                for h in range(H):
                    nc.vector.tensor_scalar(
                        out=attn16[:, HD * h : HD * h + HD],
                        in0=gs[:, 128 * h : 128 * h + HD],
                        scalar1=rec[:, h : h + 1],
                        scalar2=None,
                        op0=mybir.AluOpType.mult,
                    )
                attnT = outp.tile([128, CT, NG], f16, tag="attnT")
                pe_transpose_blocks(attn16, attnT[:].unsqueeze(3).rearrange("p u n o -> p u (n o)"), 0, rows=NG)

                out_sb = outp.tile([NG, C], f32, tag="outsb")
                for half in range(2):
                    nsl = slice(384 * half, 384 * half + 384)
                    op = ps_a.tile([NG, 384], f32, tag="psa")
                    for u_c in range(CT):
                        nc.tensor.matmul(
                            op[:],
                            attnT[:, u_c, :],
                            woT_h[:, u_c, nsl],
                            start=(u_c == 0),
                            stop=False,
                        )
                    nc.tensor.matmul(
                        op[:], ones_row[:], bo_sb[:, nsl], start=False, stop=True
                    )
                    nc.scalar.copy(out=out_sb[:, nsl], in_=op[:])
                nc.gpsimd.dma_start(out=out_d[b], in_=out_sb[:])

    nc.finalize()
    return nc


_NC_CACHE = {}


def _get_nc(b_sh, S):
    key = (b_sh, S)
    if key not in _NC_CACHE:
        _NC_CACHE[key] = build_nc(b_sh, S)
    return _NC_CACHE[key]


def kernel(query, key_in, Wq, Wk, Wv, Wo, bo):
    from concourse.bass_utils import run_bass_kernel_spmd

    query = np.ascontiguousarray(np.asarray(query, dtype=np.float32))
    key_in = np.ascontiguousarray(np.asarray(key_in, dtype=np.float32))
    Wq = np.ascontiguousarray(np.asarray(Wq, dtype=np.float32))
    Wk = np.ascontiguousarray(np.asarray(Wk, dtype=np.float32))
    Wv = np.ascontiguousarray(np.asarray(Wv, dtype=np.float32))
    Wo = np.ascontiguousarray(np.asarray(Wo, dtype=np.float32))
    bo = np.ascontiguousarray(np.asarray(bo, dtype=np.float32))

    B, _, _ = query.shape
    S = key_in.shape[1]
    n_cores = 8
    b_sh = B // n_cores
    nc = _get_nc(b_sh, S)

    in_maps = []
    for i in range(n_cores):
        bs = slice(i * b_sh, (i + 1) * b_sh)
        in_maps.append(
            {
                "query": np.ascontiguousarray(query[bs]),
                "key_in": np.ascontiguousarray(key_in[bs]),
                "Wq": Wq,
                "Wk": Wk,
                "Wv": Wv,
                "Wo": Wo,
                "bo": bo,
            }
        )
    res = run_bass_kernel_spmd(nc, in_maps, core_ids=list(range(n_cores)))
    out = np.concatenate([res.results[i]["out"] for i in range(n_cores)], axis=0)
    return out.astype(np.float32)


if __name__ == "__main__":
    nc = build_nc(1, 512)
    print("built ok")


# revision 29
# speedup vs baseline: 1.0132x; 1.0025x over previous
"""AssignAttention (hard-routing slot attention) Trainium2 kernel, 8-core data-parallel.

Problem: B=16, N=64 groups, S=4096 tokens, C=768, H=8 heads, HD=96.
  q = query @ Wq.T; k = key @ Wk.T; v = key @ Wv.T (per-head split)
  logits = q @ k.T; hard-argmax over the 64 groups per token -> one-hot
  (softmax and the *SCALE factor are argmax-invariant, so both are skipped);
  attn = onehot / (count + 1); out = (attn @ v per head) @ Wo.T + bo

Sharding: data-parallel over batch B: 16 batches / 8 cores = 2 per core.
No collectives; the host concatenates per-core outputs.

Algorithm per core (validated vs the fp32 reference: rel_l2 ~2e-3, the
residual being argmax flips on near-ties that any reimplementation incurs):
  - The logits are REASSOCIATED: instead of projecting k = key @ Wk.T
    (the dominant 8192x768x768 matmul) and then contracting with q over
    head_dim, we precompute Y[c, (h,n)] = sum_d Wk[d(head h), c] q[n, d]
    (tiny: 768x512 per batch) and compute logits[s, (h,n)] =
    sum_c keyT[c, s] Y[c, (h,n)] -- one 768-contraction matmul produces all
    8 heads' logits, and the k-projection disappears entirely.
  - Precision on the argmax path: fp16 hi/lo split x3 matmuls
    (K@Y ~= Kh@Yh + Kh@Yl + Kl@Yh, fp32 PSUM accumulation, error ~2^-21);
    q-projection the same; Y itself fp32. Head dim is zero-padded 96->128
    because PSUM accumulation groups cannot mix matmul row offsets on HW.
  - argmax via row-max + (x >= max) compare (ties are ~1-ulp rare and only
    perturb one group's mean); counts via an extra N=1 ones-column matmul
    into the same PSUM accumulator; renorm = per-partition reciprocal.
  - keyT (c-major) via PE transpose-mode (fp32, 3 blocks batched per PSUM
    bank, single strided ACT copy out); fp16 halves derived on DVE.
  - v-projection, group-sum, Wo: single-pass fp16 (linear-path error ~5e-4);
    bias via a K=1 fp32 outer-product matmul into the same PSUM group.
  - One accumulation group per 2KB PSUM bank (start zeroes the whole bank).
  - Engine split: PE does matmuls/transposes; DVE does max/is_ge/casts;
    ACT (scalar) does PSUM->SBUF copies; SWDGE does bulk key DMA; HWDGE the
    rest. Measured ~645 us on silicon at 2.4 GHz (~1.36M PE cycles,
    TensorEngine ~88% busy; clock-state dependent).
"""

import sys

if "/opt/trn_rl_repo" not in sys.path:
    sys.path.insert(0, "/opt/trn_rl_repo")

import numpy as np

import concourse.bass as bass
import concourse.mybir as mybir
from concourse import bacc
import concourse.tile as tile
from concourse.masks import make_identity

f32 = mybir.dt.float32
f16 = mybir.dt.float16

C = 768
H = 8
HD = 96
NG = 64  # groups
CT = C // 128  # 6 c-tiles
S_CHUNK = 256


def build_nc(b_sh=2, S=4096):
    nc = bacc.Bacc()

    query_d = nc.declare_dram_parameter("query", [b_sh, NG, C], f32, isOutput=False)
    key_d = nc.declare_dram_parameter("key_in", [b_sh, S, C], f32, isOutput=False)
    wq_d = nc.declare_dram_parameter("Wq", [C, C], f32, isOutput=False)
    wk_d = nc.declare_dram_parameter("Wk", [C, C], f32, isOutput=False)
    wv_d = nc.declare_dram_parameter("Wv", [C, C], f32, isOutput=False)
    wo_d = nc.declare_dram_parameter("Wo", [C, C], f32, isOutput=False)
    bo_d = nc.declare_dram_parameter("bo", [C], f32, isOutput=False)
    out_d = nc.declare_dram_parameter("out", [b_sh, NG, C], f32, isOutput=True)

    n_chunks = S // S_CHUNK
    n_sub = S_CHUNK // 128  # s-subtiles per chunk

    with tile.TileContext(nc) as tc:
        with (
            tc.tile_pool(name="wconst", bufs=1) as wconst,
            tc.tile_pool(name="qpool", bufs=1) as qpool,
            tc.tile_pool(name="kin", bufs=3) as kin,
            tc.tile_pool(name="keyT", bufs=2) as keyTp,
            tc.tile_pool(name="vp", bufs=3) as vp,
            tc.tile_pool(name="ohp", bufs=3) as ohp,
            tc.tile_pool(name="mxp", bufs=3) as mxp,
            tc.tile_pool(name="outp", bufs=1) as outp,
            tc.tile_pool(name="ps_a", bufs=2, space="PSUM") as ps_a,
            tc.tile_pool(name="ps_v", bufs=2, space="PSUM") as ps_v,
            tc.tile_pool(name="ps_tr", bufs=2, space="PSUM") as ps_tr,
            tc.tile_pool(name="ps_gs", bufs=1, space="PSUM") as ps_gs,
        ):
            # ---- constants ----
            ident64_16 = wconst.tile([NG, NG], f16)
            make_identity(nc, ident64_16[:])
            ident64_32 = wconst.tile([NG, NG], f32)
            make_identity(nc, ident64_32[:])
            ident128_16 = wconst.tile([128, 128], f16)
            make_identity(nc, ident128_16[:])
            ident128_32 = wconst.tile([128, 128], f32)
            make_identity(nc, ident128_32[:])
            ones_col = wconst.tile([128, 1], f16)
            nc.vector.memset(ones_col[:], 1.0)
            ones_row = wconst.tile([1, NG], f32)
            nc.vector.memset(ones_row[:], 1.0)
            bo_sb = wconst.tile([1, C], f32)
            nc.sync.dma_start(out=bo_sb[:], in_=bo_d[:].unsqueeze(0))

            def pe_transpose_blocks_f32(src, dst, t):
                """PE-transpose 6 f32 [128,128] blocks src[:, 128u:128u+128]
                into dst[:, u, 128t:128t+128]; 3 blocks per PSUM bank."""
                for g in range(2):
                    trp = ps_tr.tile([128, 3, 128], f32, tag="pstr")
                    for j in range(3):
                        u = 3 * g + j
                        nc.tensor.matmul(
                            trp[:, j, :],
                            src[:, 128 * u : 128 * u + 128],
                            ident128_32[:],
                            is_transpose=True,
                            start=(j == 0),
                            stop=(j == 2),
                        )
                    nc.scalar.copy(
                        out=dst[:, 3 * g : 3 * g + 3, 128 * t : 128 * t + 128],
                        in_=trp[:],
                    )

            def pe_transpose_blocks(src, dst, t, rows=128):
                """PE-transpose 6 f16 [rows,128] blocks src[:, 128u:128u+128]
                (u=0..5) into dst[:, u, 128t:128t+rows] via one batched PSUM
                bank + a single strided ACT copy."""
                ident = ident128_16 if rows == 128 else ident64_16
                trp = ps_tr.tile([128, CT, rows], f16, tag="pstr")
                for u in range(CT):
                    nc.tensor.matmul(
                        trp[:, u, :],
                        src[0:rows, 128 * u : 128 * u + 128],
                        ident[:],
                        is_transpose=True,
                        start=(u == 0),
                        stop=(u == CT - 1),
                    )
                nc.scalar.copy(out=dst[:, :, 128 * t : 128 * t + rows], in_=trp[:])

            # ---- weight prep: transpose to c-major fp16 hi/lo ----
            # wT[p, u, d] = W[d, 128u + p]
            CP = 128 * H  # d-padded width for Wq/Wk (head h at 128h..128h+96)
            wqT_h = wconst.tile([128, CT, CP], f16)
            wqT_l = wconst.tile([128, CT, CP], f16)
            # Wk kept NATURAL fp32 (d-padded rows, c free) for the
            # Y = Wk_h^T q_h precompute; logits = keyT^T @ Y in fp32
            # (~2.1 cyc/row warm measured, and exact precision)
            wk_nat = wconst.tile([128, H, C], f32)
            wvT_h = wconst.tile([128, CT, C], f16)
            woT_h = wconst.tile([128, CT, C], f16)

            # Prefetch the first key chunk so the PE has transpose work
            # while the weight tiles stream in.
            knat_pre = kin.tile([128, n_sub, C], f32, tag="knat")
            nc.gpsimd.dma_start(
                out=knat_pre[:],
                in_=key_d[0, 0:S_CHUNK, :].rearrange("(i p) c -> p i c", p=128),
            )

            # Wq/Wk use the d-padded layout (head h -> cols 128h..128h+96, rest
            # zero) so every logits matmul is a single offset-0 K=128 matmul
            # (mixed-row-offset PSUM accumulation groups fail to load on HW).
            wtmp_ctx = tc.tile_pool(name="wtmp", bufs=2)
            wtmp = wtmp_ctx.__enter__()
            for hd in range(H):
                wnat = wtmp.tile([128, C], f32, tag="wnat")
                nc.vector.memset(wnat[96:128, :], 0.0)
                nc.sync.dma_start(
                    out=wnat[0:HD, :], in_=wq_d[HD * hd : HD * hd + HD, :]
                )
                whi = wtmp.tile([128, C], f16, tag="whi")
                nc.vector.tensor_copy(whi[:], wnat[:])
                pe_transpose_blocks(whi[:], wqT_h[:], hd)
                wlo = wtmp.tile([128, C], f16, tag="wlo")
                nc.vector.tensor_tensor(
                    out=wlo[:], in0=wnat[:], in1=whi[:], op=mybir.AluOpType.subtract
                )
                pe_transpose_blocks(wlo[:], wqT_l[:], hd)
            nc.vector.memset(wk_nat[96:128, :, :], 0.0)
            for hd in range(H):
                nc.sync.dma_start(
                    out=wk_nat[0:HD, hd, :], in_=wk_d[HD * hd : HD * hd + HD, :]
                )
            for w_dram, dst_h in ((wv_d, wvT_h), (wo_d, woT_h)):
                for t in range(CT):
                    wnat = wtmp.tile([128, C], f32, tag="wnat")
                    nc.sync.dma_start(out=wnat[:], in_=w_dram[128 * t : 128 * t + 128, :])
                    whi = wtmp.tile([128, C], f16, tag="whi")
                    nc.vector.tensor_copy(whi[:], wnat[:])
                    pe_transpose_blocks(whi[:], dst_h[:], t)
            wtmp_ctx.__exit__(None, None, None)

            for b in range(b_sh):
                # ---- Q path ----
                q_nat = qpool.tile([NG, C], f32, tag="qnat")
                nc.sync.dma_start(out=q_nat[:], in_=query_d[b])
                qh_nat = qpool.tile([NG, C], f16, tag="qhnat")
                ql_nat = qpool.tile([NG, C], f16, tag="qlnat")
                nc.vector.tensor_copy(qh_nat[:], q_nat[:])
                nc.vector.tensor_tensor(
                    out=ql_nat[:], in0=q_nat[:], in1=qh_nat[:], op=mybir.AluOpType.subtract
                )
                # queryT (c-major) fp16 halves via DMA xbar ([64,128] blocks)
                qTq_h = qpool.tile([128, CT, NG], f16, tag="qTqh")
                qTq_l = qpool.tile([128, CT, NG], f16, tag="qTql")
                for qsrc, dst in ((qh_nat, qTq_h), (ql_nat, qTq_l)):
                    pe_transpose_blocks(qsrc, dst[:].unsqueeze(3).rearrange("p u n o -> p u (n o)"), 0, rows=NG)
                # q projection (natural layout, M=64), d-padded: q_pad [64, 1024]
                q_sb = qpool.tile([NG, CP], f32, tag="qsb")
                for half in range(2):
                    nsl = slice(512 * half, 512 * half + 512)
                    qp = ps_a.tile([NG, 512], f32, tag="psa")
                    first = True
                    for u in range(CT):
                        for lhsT, rhs in (
                            (qTq_h, wqT_h),
                            (qTq_h, wqT_l),
                            (qTq_l, wqT_h),
                        ):
                            nc.tensor.matmul(
                                qp[:],
                                lhsT[:, u, :],
                                rhs[:, u, nsl],
                                start=first,
                                stop=(u == CT - 1 and lhsT is qTq_l),
                            )
                            first = False
                    nc.scalar.copy(out=q_sb[:, nsl], in_=qp[:])
                # qT (padded d-major, per head) fp32 via PE transpose
                qT = qpool.tile([128, H, NG], f32, tag="qT")
                for hd in range(H):
                    trq2 = ps_a.tile([128, NG], f32, tag="psa")
                    nc.tensor.matmul(
                        trq2[:],
                        q_sb[:, 128 * hd : 128 * hd + 128],
                        ident64_32[:],
                        is_transpose=True,
                        start=True,
                        stop=True,
                    )
                    nc.scalar.copy(out=qT[:, hd, :], in_=trq2[:])
                # Y_all[c, 64h+n] = sum_d Wk[d(head h), c] * q[n, d], fp32,
                # then fp16 hi/lo split. logits = keyT^T @ Y_all (split x3).
                Y_h = qpool.tile([128, CT, 8 * NG], f16, tag="Yh")
                Y_l = qpool.tile([128, CT, 8 * NG], f16, tag="Yl")
                for u_c in range(CT):
                    yp = ps_a.tile([128, 8 * NG], f32, tag="psa")
                    csl = slice(128 * u_c, 128 * u_c + 128)
                    for hd in range(H):
                        nc.tensor.matmul(
                            yp[:, NG * hd : NG * hd + NG],
                            wk_nat[:, hd, csl],
                            qT[:, hd, :],
                            start=(hd == 0),
                            stop=(hd == H - 1),
                        )
                    nc.vector.tensor_copy(Y_h[:, u_c, :], yp[:])
                    nc.vector.tensor_tensor(
                        out=Y_l[:, u_c, :], in0=yp[:], in1=Y_h[:, u_c, :],
                        op=mybir.AluOpType.subtract,
                    )

                # ---- group-sum accumulator for this b ----
                gs = ps_gs.tile([NG, 8 * 128], f32, tag="gs")

                for chunk in range(n_chunks):
                    s0 = chunk * S_CHUNK
                    # load key chunk (chunk 0 of b 0 was prefetched)
                    if b == 0 and chunk == 0:
                        knat = knat_pre
                    else:
                        knat = kin.tile([128, n_sub, C], f32, tag="knat")
                        nc.gpsimd.dma_start(
                            out=knat[:],
                            in_=key_d[b, s0 : s0 + S_CHUNK, :].rearrange(
                                "(i p) c -> p i c", p=128
                            ),
                        )
                    # keyT via fp32 PE transposes, then fp16 hi/lo split
                    # (hi also feeds the v-projection)
                    keyT = keyTp.tile([128, CT, S_CHUNK], f32, tag="keyT")
                    for i in range(n_sub):
                        pe_transpose_blocks_f32(knat[:, i, :], keyT[:], i)
                    kTh = keyTp.tile([128, CT, S_CHUNK], f16, tag="kTh")
                    kTl = keyTp.tile([128, CT, S_CHUNK], f16, tag="kTl")
                    nc.vector.tensor_copy(kTh[:], keyT[:])
                    nc.vector.tensor_tensor(
                        out=kTl[:], in0=keyT[:], in1=kTh[:], op=mybir.AluOpType.subtract
                    )

                    for i in range(n_sub):
                        ssl = slice(128 * i, 128 * i + 128)
                        # v projection (natural [s, d]) fp16 single pass;
                        # two single-bank PSUM tiles so copies pipeline
                        vpsA = ps_v.tile([128, 384], f32, tag="vps")
                        vpsB = ps_v.tile([128, 384], f32, tag="vps")
                        for u_c in range(CT):
                            nc.tensor.matmul(
                                vpsA[:],
                                kTh[:, u_c, ssl],
                                wvT_h[:, u_c, 0:384],
                                start=(u_c == 0),
                                stop=(u_c == CT - 1),
                            )
                            nc.tensor.matmul(
                                vpsB[:],
                                kTh[:, u_c, ssl],
                                wvT_h[:, u_c, 384:768],
                                start=(u_c == 0),
                                stop=(u_c == CT - 1),
                            )
                        v16 = vp.tile([128, C], f16, tag="v16")
                        nc.scalar.copy(out=v16[:, 0:384], in_=vpsA[:])
                        nc.scalar.copy(out=v16[:, 384:768], in_=vpsB[:])

                        # logits for all 8 heads at once: lg[s, 64h+n] =
                        # sum_c keyT[c, s] Y_all[c, 64h+n], fp16 split x3.
                        # One accumulation group per PSUM bank: start only on
                        # the first matmul (zeroes the 2KB region), stop last.
                        lg = ps_a.tile([128, 8 * NG], f32, tag="psa")
                        first = True
                        for u_c in range(CT):
                            for kt, yt in ((kTh, Y_h), (kTh, Y_l), (kTl, Y_h)):
                                nc.tensor.matmul(
                                    lg[:],
                                    kt[:, u_c, ssl],
                                    yt[:, u_c, :],
                                    start=first,
                                    stop=(u_c == CT - 1 and kt is kTl),
                                )
                                first = False
                        # argmax -> one-hot via (x >= rowmax), fp16
                        mx = mxp.tile([128, H], f32, tag="mx")
                        lg3 = lg[:].rearrange("p (h n) -> p h n", h=H)
                        nc.vector.tensor_reduce(
                            out=mx[:],
                            in_=lg3,
                            axis=mybir.AxisListType.X,
                            op=mybir.AluOpType.max,
                        )
                        oh = ohp.tile([128, H * NG], f16, tag="oh")
                        nc.vector.tensor_tensor(
                            out=oh[:].rearrange("p (h n) -> p h n", h=H),
                            in0=lg3,
                            in1=mx[:].unsqueeze(2).to_broadcast((128, H, NG)),
                            op=mybir.AluOpType.is_ge,
                        )

                        # group sums + counts (fp16 matmuls, fp32 accum)
                        # gs spans 2 PSUM banks (heads 0-3, heads 4-7): one
                        # start per bank (zeroes the 2KB region), one stop
                        last = chunk == n_chunks - 1 and i == n_sub - 1
                        first = chunk == 0 and i == 0
                        for h in range(H):
                            lh = oh[:, NG * h : NG * h + NG]
                            nc.tensor.matmul(
                                gs[:, 128 * h : 128 * h + HD],
                                lh,
                                v16[:, HD * h : HD * h + HD],
                                start=(first and h in (0, 4)),
                                stop=False,
                            )
                            nc.tensor.matmul(
                                gs[:, 128 * h + HD : 128 * h + HD + 1],
                                lh,
                                ones_col[:],
                                start=False,
                                stop=(last and h in (3, 7)),
                            )

                # ---- finalize b: divide by (count+1), transpose, Wo, bias ----
                cnt = outp.tile([NG, H], f32, tag="cnt")
                nc.vector.tensor_scalar(
                    out=cnt[:],
                    in0=gs[:].rearrange("p (h q) -> p h q", q=128)[:, :, HD],
                    scalar1=1.0,
                    scalar2=None,
                    op0=mybir.AluOpType.add,
                )
                rec = outp.tile([NG, H], f32, tag="rec")
                nc.vector.reciprocal(rec[:], cnt[:])
                attn16 = outp.tile([NG, C], f16, tag="attn16")
                for h in range(H):
                    nc.vector.tensor_scalar(
                        out=attn16[:, HD * h : HD * h + HD],
                        in0=gs[:, 128 * h : 128 * h + HD],
                        scalar1=rec[:, h : h + 1],
                        scalar2=None,
                        op0=mybir.AluOpType.mult,
                    )
                attnT = outp.tile([128, CT, NG], f16, tag="attnT")
                pe_transpose_blocks(attn16, attnT[:].unsqueeze(3).rearrange("p u n o -> p u (n o)"), 0, rows=NG)

                out_sb = outp.tile([NG, C], f32, tag="outsb")
                for half in range(2):
                    nsl = slice(384 * half, 384 * half + 384)
                    op = ps_a.tile([NG, 384], f32, tag="psa")
                    for u_c in range(CT):
                        nc.tensor.matmul(
                            op[:],
                            attnT[:, u_c, :],
                            woT_h[:, u_c, nsl],
                            start=(u_c == 0),
                            stop=False,
                        )
                    nc.tensor.matmul(
                        op[:], ones_row[:], bo_sb[:, nsl], start=False, stop=True
                    )
                    nc.scalar.copy(out=out_sb[:, nsl], in_=op[:])
                nc.gpsimd.dma_start(out=out_d[b], in_=out_sb[:])

    nc.finalize()
    return nc


_NC_CACHE = {}


def _get_nc(b_sh, S):
    key = (b_sh, S)
    if key not in _NC_CACHE:
        _NC_CACHE[key] = build_nc(b_sh, S)
    return _NC_CACHE[key]


def kernel(query, key_in, Wq, Wk, Wv, Wo, bo):
    from concourse.bass_utils import run_bass_kernel_spmd

    query = np.ascontiguousarray(np.asarray(query, dtype=np.float32))
    key_in = np.ascontiguousarray(np.asarray(key_in, dtype=np.float32))
    Wq = np.ascontiguousarray(np.asarray(Wq, dtype=np.float32))
    Wk = np.ascontiguousarray(np.asarray(Wk, dtype=np.float32))
    Wv = np.ascontiguousarray(np.asarray(Wv, dtype=np.float32))
    Wo = np.ascontiguousarray(np.asarray(Wo, dtype=np.float32))
    bo = np.ascontiguousarray(np.asarray(bo, dtype=np.float32))

    B, _, _ = query.shape
    S = key_in.shape[1]
    n_cores = 8
    b_sh = B // n_cores
    nc = _get_nc(b_sh, S)

    in_maps = []
    for i in range(n_cores):
        bs = slice(i * b_sh, (i + 1) * b_sh)
        in_maps.append(
            {
                "query": np.ascontiguousarray(query[bs]),
                "key_in": np.ascontiguousarray(key_in[bs]),
                "Wq": Wq,
                "Wk": Wk,
                "Wv": Wv,
                "Wo": Wo,
                "bo": bo,
            }
        )
    res = run_bass_kernel_spmd(nc, in_maps, core_ids=list(range(n_cores)))
    out = np.concatenate([res.results[i]["out"] for i in range(n_cores)], axis=0)
    return out.astype(np.float32)


if __name__ == "__main__":
    nc = build_nc(1, 512)
    print("built ok")


# revision 31
# speedup vs baseline: 1.0338x; 1.0204x over previous
"""AssignAttention (hard-routing slot attention) Trainium2 kernel, 8-core data-parallel.

Problem: B=16, N=64 groups, S=4096 tokens, C=768, H=8 heads, HD=96.
  q = query @ Wq.T; k = key @ Wk.T; v = key @ Wv.T (per-head split)
  logits = q @ k.T; hard-argmax over the 64 groups per token -> one-hot
  (softmax and the *SCALE factor are argmax-invariant, so both are skipped);
  attn = onehot / (count + 1); out = (attn @ v per head) @ Wo.T + bo

Sharding: data-parallel over batch B: 16 batches / 8 cores = 2 per core.
No collectives; the host concatenates per-core outputs.

Algorithm per core (validated vs the fp32 reference: rel_l2 ~2e-3, the
residual being argmax flips on near-ties that any reimplementation incurs):
  - The logits are REASSOCIATED: instead of projecting k = key @ Wk.T
    (the dominant 8192x768x768 matmul) and then contracting with q over
    head_dim, we precompute Y[c, (h,n)] = sum_d Wk[d(head h), c] q[n, d]
    (tiny: 768x512 per batch) and compute logits[s, (h,n)] =
    sum_c keyT[c, s] Y[c, (h,n)] -- one 768-contraction matmul produces all
    8 heads' logits, and the k-projection disappears entirely.
  - Precision on the argmax path: fp16 hi/lo split x3 matmuls
    (K@Y ~= Kh@Yh + Kh@Yl + Kl@Yh, fp32 PSUM accumulation, error ~2^-21);
    q-projection the same; Y itself fp32. Head dim is zero-padded 96->128
    because PSUM accumulation groups cannot mix matmul row offsets on HW.
  - argmax via row-max + (x >= max) compare (ties are ~1-ulp rare and only
    perturb one group's mean); counts via an extra N=1 ones-column matmul
    into the same PSUM accumulator; renorm = per-partition reciprocal.
  - keyT (c-major) via PE transpose-mode (fp32, 3 blocks batched per PSUM
    bank, single strided ACT copy out); fp16 halves derived on DVE.
  - v-projection, group-sum, Wo: single-pass fp16 (linear-path error ~5e-4);
    bias via a K=1 fp32 outer-product matmul into the same PSUM group.
  - One accumulation group per 2KB PSUM bank (start zeroes the whole bank).
  - Engine split: PE does matmuls/transposes; DVE does max/is_ge/casts;
    ACT (scalar) does PSUM->SBUF copies; SWDGE does bulk key DMA; HWDGE the
    rest. Measured ~645 us on silicon at 2.4 GHz (~1.36M PE cycles,
    TensorEngine ~88% busy; clock-state dependent).
"""

import sys

if "/opt/trn_rl_repo" not in sys.path:
    sys.path.insert(0, "/opt/trn_rl_repo")

import numpy as np

import concourse.bass as bass
import concourse.mybir as mybir
from concourse import bacc
import concourse.tile as tile
from concourse.masks import make_identity

f32 = mybir.dt.float32
f16 = mybir.dt.float16

C = 768
H = 8
HD = 96
NG = 64  # groups
CT = C // 128  # 6 c-tiles
S_CHUNK = 256


def build_nc(b_sh=2, S=4096):
    nc = bacc.Bacc()

    query_d = nc.declare_dram_parameter("query", [b_sh, NG, C], f32, isOutput=False)
    key_d = nc.declare_dram_parameter("key_in", [b_sh, S, C], f32, isOutput=False)
    wq_d = nc.declare_dram_parameter("Wq", [C, C], f32, isOutput=False)
    wk_d = nc.declare_dram_parameter("Wk", [C, C], f32, isOutput=False)
    wv_d = nc.declare_dram_parameter("Wv", [C, C], f32, isOutput=False)
    wo_d = nc.declare_dram_parameter("Wo", [C, C], f32, isOutput=False)
    bo_d = nc.declare_dram_parameter("bo", [C], f32, isOutput=False)
    out_d = nc.declare_dram_parameter("out", [b_sh, NG, C], f32, isOutput=True)

    n_chunks = S // S_CHUNK
    n_sub = S_CHUNK // 128  # s-subtiles per chunk

    with tile.TileContext(nc) as tc:
        with (
            tc.tile_pool(name="wconst", bufs=1) as wconst,
            tc.tile_pool(name="qpool", bufs=1) as qpool,
            tc.tile_pool(name="kin", bufs=3) as kin,
            tc.tile_pool(name="keyT", bufs=3) as keyTp,
            tc.tile_pool(name="vp", bufs=4) as vp,
            tc.tile_pool(name="ohp", bufs=4) as ohp,
            tc.tile_pool(name="mxp", bufs=3) as mxp,
            tc.tile_pool(name="outp", bufs=1) as outp,
            tc.tile_pool(name="ps_a", bufs=2, space="PSUM") as ps_a,
            tc.tile_pool(name="ps_v", bufs=2, space="PSUM") as ps_v,
            tc.tile_pool(name="ps_tr", bufs=2, space="PSUM") as ps_tr,
            tc.tile_pool(name="ps_gs", bufs=1, space="PSUM") as ps_gs,
        ):
            # ---- constants ----
            ident64_16 = wconst.tile([NG, NG], f16)
            make_identity(nc, ident64_16[:])
            ident64_32 = wconst.tile([NG, NG], f32)
            make_identity(nc, ident64_32[:])
            ident128_16 = wconst.tile([128, 128], f16)
            make_identity(nc, ident128_16[:])
            ident128_32 = wconst.tile([128, 128], f32)
            make_identity(nc, ident128_32[:])
            ones_col = wconst.tile([128, 1], f16)
            nc.vector.memset(ones_col[:], 1.0)
            ones_row = wconst.tile([1, NG], f32)
            nc.vector.memset(ones_row[:], 1.0)
            bo_sb = wconst.tile([1, C], f32)
            nc.sync.dma_start(out=bo_sb[:], in_=bo_d[:].unsqueeze(0))

            def pe_transpose_blocks_f32(src, dst, t):
                """PE-transpose 6 f32 [128,128] blocks src[:, 128u:128u+128]
                into dst[:, u, 128t:128t+128]; 3 blocks per PSUM bank."""
                for g in range(2):
                    trp = ps_tr.tile([128, 3, 128], f32, tag="pstr")
                    for j in range(3):
                        u = 3 * g + j
                        nc.tensor.matmul(
                            trp[:, j, :],
                            src[:, 128 * u : 128 * u + 128],
                            ident128_32[:],
                            is_transpose=True,
                            start=(j == 0),
                            stop=(j == 2),
                        )
                    nc.scalar.copy(
                        out=dst[:, 3 * g : 3 * g + 3, 128 * t : 128 * t + 128],
                        in_=trp[:],
                    )

            def pe_transpose_blocks(src, dst, t, rows=128):
                """PE-transpose 6 f16 [rows,128] blocks src[:, 128u:128u+128]
                (u=0..5) into dst[:, u, 128t:128t+rows] via one batched PSUM
                bank + a single strided ACT copy."""
                ident = ident128_16 if rows == 128 else ident64_16
                trp = ps_tr.tile([128, CT, rows], f16, tag="pstr")
                for u in range(CT):
                    nc.tensor.matmul(
                        trp[:, u, :],
                        src[0:rows, 128 * u : 128 * u + 128],
                        ident[:],
                        is_transpose=True,
                        start=(u == 0),
                        stop=(u == CT - 1),
                    )
                nc.scalar.copy(out=dst[:, :, 128 * t : 128 * t + rows], in_=trp[:])

            # ---- weight prep: transpose to c-major fp16 hi/lo ----
            # wT[p, u, d] = W[d, 128u + p]
            CP = 128 * H  # d-padded width for Wq/Wk (head h at 128h..128h+96)
            wqT_h = wconst.tile([128, CT, CP], f16)
            wqT_l = wconst.tile([128, CT, CP], f16)
            # Wk kept NATURAL fp32 (d-padded rows, c free) for the
            # Y = Wk_h^T q_h precompute; logits = keyT^T @ Y in fp32
            # (~2.1 cyc/row warm measured, and exact precision)
            wk_nat = wconst.tile([128, H, C], f32)
            wvT_h = wconst.tile([128, CT, C], f16)
            woT_h = wconst.tile([128, CT, C], f16)

            # Prefetch the first key chunk so the PE has transpose work
            # while the weight tiles stream in.
            knat_pre = kin.tile([128, n_sub, C], f32, tag="knat")
            nc.gpsimd.dma_start(
                out=knat_pre[:],
                in_=key_d[0, 0:S_CHUNK, :].rearrange("(i p) c -> p i c", p=128),
            )

            # Wq/Wk use the d-padded layout (head h -> cols 128h..128h+96, rest
            # zero) so every logits matmul is a single offset-0 K=128 matmul
            # (mixed-row-offset PSUM accumulation groups fail to load on HW).
            wtmp_ctx = tc.tile_pool(name="wtmp", bufs=4)
            wtmp = wtmp_ctx.__enter__()
            for hd in range(H):
                wnat = wtmp.tile([128, C], f32, tag="wnat")
                nc.vector.memset(wnat[96:128, :], 0.0)
                nc.sync.dma_start(
                    out=wnat[0:HD, :], in_=wq_d[HD * hd : HD * hd + HD, :]
                )
                whi = wtmp.tile([128, C], f16, tag="whi")
                nc.vector.tensor_copy(whi[:], wnat[:])
                pe_transpose_blocks(whi[:], wqT_h[:], hd)
                wlo = wtmp.tile([128, C], f16, tag="wlo")
                nc.vector.tensor_tensor(
                    out=wlo[:], in0=wnat[:], in1=whi[:], op=mybir.AluOpType.subtract
                )
                pe_transpose_blocks(wlo[:], wqT_l[:], hd)
            nc.vector.memset(wk_nat[96:128, :, :], 0.0)
            for hd in range(H):
                nc.sync.dma_start(
                    out=wk_nat[0:HD, hd, :], in_=wk_d[HD * hd : HD * hd + HD, :]
                )
            for w_dram, dst_h in ((wv_d, wvT_h), (wo_d, woT_h)):
                for t in range(CT):
                    wnat = wtmp.tile([128, C], f32, tag="wnat")
                    nc.sync.dma_start(out=wnat[:], in_=w_dram[128 * t : 128 * t + 128, :])
                    whi = wtmp.tile([128, C], f16, tag="whi")
                    nc.vector.tensor_copy(whi[:], wnat[:])
                    pe_transpose_blocks(whi[:], dst_h[:], t)
            wtmp_ctx.__exit__(None, None, None)

            for b in range(b_sh):
                # ---- Q path ----
                q_nat = qpool.tile([NG, C], f32, tag="qnat")
                nc.sync.dma_start(out=q_nat[:], in_=query_d[b])
                qh_nat = qpool.tile([NG, C], f16, tag="qhnat")
                ql_nat = qpool.tile([NG, C], f16, tag="qlnat")
                nc.vector.tensor_copy(qh_nat[:], q_nat[:])
                nc.vector.tensor_tensor(
                    out=ql_nat[:], in0=q_nat[:], in1=qh_nat[:], op=mybir.AluOpType.subtract
                )
                # queryT (c-major) fp16 halves via DMA xbar ([64,128] blocks)
                qTq_h = qpool.tile([128, CT, NG], f16, tag="qTqh")
                qTq_l = qpool.tile([128, CT, NG], f16, tag="qTql")
                for qsrc, dst in ((qh_nat, qTq_h), (ql_nat, qTq_l)):
                    pe_transpose_blocks(qsrc, dst[:].unsqueeze(3).rearrange("p u n o -> p u (n o)"), 0, rows=NG)
                # q projection (natural layout, M=64), d-padded: q_pad [64, 1024]
                q_sb = qpool.tile([NG, CP], f32, tag="qsb")
                for half in range(2):
                    nsl = slice(512 * half, 512 * half + 512)
                    qp = ps_a.tile([NG, 512], f32, tag="psa")
                    first = True
                    for u in range(CT):
                        for lhsT, rhs in (
                            (qTq_h, wqT_h),
                            (qTq_h, wqT_l),
                            (qTq_l, wqT_h),
                        ):
                            nc.tensor.matmul(
                                qp[:],
                                lhsT[:, u, :],
                                rhs[:, u, nsl],
                                start=first,
                                stop=(u == CT - 1 and lhsT is qTq_l),
                            )
                            first = False
                    nc.scalar.copy(out=q_sb[:, nsl], in_=qp[:])
                # qT (padded d-major, per head) fp32 via PE transpose
                qT = qpool.tile([128, H, NG], f32, tag="qT")
                for hd in range(H):
                    trq2 = ps_a.tile([128, NG], f32, tag="psa")
                    nc.tensor.matmul(
                        trq2[:],
                        q_sb[:, 128 * hd : 128 * hd + 128],
                        ident64_32[:],
                        is_transpose=True,
                        start=True,
                        stop=True,
                    )
                    nc.scalar.copy(out=qT[:, hd, :], in_=trq2[:])
                # Y_all[c, 64h+n] = sum_d Wk[d(head h), c] * q[n, d], fp32,
                # then fp16 hi/lo split. logits = keyT^T @ Y_all (split x3).
                Y_h = qpool.tile([128, CT, 8 * NG], f16, tag="Yh")
                Y_l = qpool.tile([128, CT, 8 * NG], f16, tag="Yl")
                for u_c in range(CT):
                    yp = ps_a.tile([128, 8 * NG], f32, tag="psa")
                    csl = slice(128 * u_c, 128 * u_c + 128)
                    for hd in range(H):
                        nc.tensor.matmul(
                            yp[:, NG * hd : NG * hd + NG],
                            wk_nat[:, hd, csl],
                            qT[:, hd, :],
                            start=(hd == 0),
                            stop=(hd == H - 1),
                        )
                    nc.vector.tensor_copy(Y_h[:, u_c, :], yp[:])
                    nc.vector.tensor_tensor(
                        out=Y_l[:, u_c, :], in0=yp[:], in1=Y_h[:, u_c, :],
                        op=mybir.AluOpType.subtract,
                    )

                # ---- group-sum accumulator for this b ----
                gs = ps_gs.tile([NG, 8 * 128], f32, tag="gs")

                for chunk in range(n_chunks):
                    s0 = chunk * S_CHUNK
                    # load key chunk (chunk 0 of b 0 was prefetched)
                    if b == 0 and chunk == 0:
                        knat = knat_pre
                    else:
                        knat = kin.tile([128, n_sub, C], f32, tag="knat")
                        nc.gpsimd.dma_start(
                            out=knat[:],
                            in_=key_d[b, s0 : s0 + S_CHUNK, :].rearrange(
                                "(i p) c -> p i c", p=128
                            ),
                        )
                    # keyT via fp32 PE transposes, then fp16 hi/lo split
                    # (hi also feeds the v-projection)
                    keyT = keyTp.tile([128, CT, S_CHUNK], f32, tag="keyT")
                    for i in range(n_sub):
                        pe_transpose_blocks_f32(knat[:, i, :], keyT[:], i)
                    kTh = keyTp.tile([128, CT, S_CHUNK], f16, tag="kTh")
                    kTl = keyTp.tile([128, CT, S_CHUNK], f16, tag="kTl")
                    nc.vector.tensor_copy(kTh[:], keyT[:])
                    nc.vector.tensor_tensor(
                        out=kTl[:], in0=keyT[:], in1=kTh[:], op=mybir.AluOpType.subtract
                    )

                    for i in range(n_sub):
                        ssl = slice(128 * i, 128 * i + 128)
                        # v projection (natural [s, d]) fp16 single pass;
                        # two single-bank PSUM tiles so copies pipeline
                        vpsA = ps_v.tile([128, 384], f32, tag="vps")
                        vpsB = ps_v.tile([128, 384], f32, tag="vps")
                        for u_c in range(CT):
                            nc.tensor.matmul(
                                vpsA[:],
                                kTh[:, u_c, ssl],
                                wvT_h[:, u_c, 0:384],
                                start=(u_c == 0),
                                stop=(u_c == CT - 1),
                            )
                            nc.tensor.matmul(
                                vpsB[:],
                                kTh[:, u_c, ssl],
                                wvT_h[:, u_c, 384:768],
                                start=(u_c == 0),
                                stop=(u_c == CT - 1),
                            )
                        v16 = vp.tile([128, C], f16, tag="v16")
                        nc.scalar.copy(out=v16[:, 0:384], in_=vpsA[:])
                        nc.scalar.copy(out=v16[:, 384:768], in_=vpsB[:])

                        # logits for all 8 heads at once: lg[s, 64h+n] =
                        # sum_c keyT[c, s] Y_all[c, 64h+n], fp16 split x3.
                        # One accumulation group per PSUM bank: start only on
                        # the first matmul (zeroes the 2KB region), stop last.
                        lg = ps_a.tile([128, 8 * NG], f32, tag="psa")
                        first = True
                        for u_c in range(CT):
                            for kt, yt in ((kTh, Y_h), (kTh, Y_l), (kTl, Y_h)):
                                nc.tensor.matmul(
                                    lg[:],
                                    kt[:, u_c, ssl],
                                    yt[:, u_c, :],
                                    start=first,
                                    stop=(u_c == CT - 1 and kt is kTl),
                                )
                                first = False
                        # argmax -> one-hot via (x >= rowmax), fp16
                        mx = mxp.tile([128, H], f32, tag="mx")
                        lg3 = lg[:].rearrange("p (h n) -> p h n", h=H)
                        nc.vector.tensor_reduce(
                            out=mx[:],
                            in_=lg3,
                            axis=mybir.AxisListType.X,
                            op=mybir.AluOpType.max,
                        )
                        oh = ohp.tile([128, H * NG], f16, tag="oh")
                        nc.vector.tensor_tensor(
                            out=oh[:].rearrange("p (h n) -> p h n", h=H),
                            in0=lg3,
                            in1=mx[:].unsqueeze(2).to_broadcast((128, H, NG)),
                            op=mybir.AluOpType.is_ge,
                        )

                        # group sums + counts (fp16 matmuls, fp32 accum)
                        # gs spans 2 PSUM banks (heads 0-3, heads 4-7): one
                        # start per bank (zeroes the 2KB region), one stop
                        last = chunk == n_chunks - 1 and i == n_sub - 1
                        first = chunk == 0 and i == 0
                        for h in range(H):
                            lh = oh[:, NG * h : NG * h + NG]
                            nc.tensor.matmul(
                                gs[:, 128 * h : 128 * h + HD],
                                lh,
                                v16[:, HD * h : HD * h + HD],
                                start=(first and h in (0, 4)),
                                stop=False,
                            )
                            nc.tensor.matmul(
                                gs[:, 128 * h + HD : 128 * h + HD + 1],
                                lh,
                                ones_col[:],
                                start=False,
                                stop=(last and h in (3, 7)),
                            )

                # ---- finalize b: divide by (count+1), transpose, Wo, bias ----
                cnt = outp.tile([NG, H], f32, tag="cnt")
                nc.vector.tensor_scalar(
                    out=cnt[:],
                    in0=gs[:].rearrange("p (h q) -> p h q", q=128)[:, :, HD],
                    scalar1=1.0,
                    scalar2=None,
                    op0=mybir.AluOpType.add,
                )
                rec = outp.tile([NG, H], f32, tag="rec")
                nc.vector.reciprocal(rec[:], cnt[:])
                attn16 = outp.tile([NG, C], f16, tag="attn16")
                for h in range(H):
                    nc.vector.tensor_scalar(
                        out=attn16[:, HD * h : HD * h + HD],
                        in0=gs[:, 128 * h : 128 * h + HD],
                        scalar1=rec[:, h : h + 1],
                        scalar2=None,
                        op0=mybir.AluOpType.mult,
                    )
                attnT = outp.tile([128, CT, NG], f16, tag="attnT")
                pe_transpose_blocks(attn16, attnT[:].unsqueeze(3).rearrange("p u n o -> p u (n o)"), 0, rows=NG)

                out_sb = outp.tile([NG, C], f32, tag="outsb")
                for half in range(2):
                    nsl = slice(384 * half, 384 * half + 384)
                    op = ps_a.tile([NG, 384], f32, tag="psa")
                    for u_c in range(CT):
                        nc.tensor.matmul(
                            op[:],
                            attnT[:, u_c, :],
                            woT_h[:, u_c, nsl],
                            start=(u_c == 0),
                            stop=False,
                        )
                    nc.tensor.matmul(
                        op[:], ones_row[:], bo_sb[:, nsl], start=False, stop=True
                    )
                    nc.scalar.copy(out=out_sb[:, nsl], in_=op[:])
                nc.gpsimd.dma_start(out=out_d[b], in_=out_sb[:])

    nc.finalize()
    return nc


_NC_CACHE = {}


def _get_nc(b_sh, S):
    key = (b_sh, S)
    if key not in _NC_CACHE:
        _NC_CACHE[key] = build_nc(b_sh, S)
    return _NC_CACHE[key]


def kernel(query, key_in, Wq, Wk, Wv, Wo, bo):
    from concourse.bass_utils import run_bass_kernel_spmd

    query = np.ascontiguousarray(np.asarray(query, dtype=np.float32))
    key_in = np.ascontiguousarray(np.asarray(key_in, dtype=np.float32))
    Wq = np.ascontiguousarray(np.asarray(Wq, dtype=np.float32))
    Wk = np.ascontiguousarray(np.asarray(Wk, dtype=np.float32))
    Wv = np.ascontiguousarray(np.asarray(Wv, dtype=np.float32))
    Wo = np.ascontiguousarray(np.asarray(Wo, dtype=np.float32))
    bo = np.ascontiguousarray(np.asarray(bo, dtype=np.float32))

    B, _, _ = query.shape
    S = key_in.shape[1]
    n_cores = 8
    b_sh = B // n_cores
    nc = _get_nc(b_sh, S)

    in_maps = []
    for i in range(n_cores):
        bs = slice(i * b_sh, (i + 1) * b_sh)
        in_maps.append(
            {
                "query": np.ascontiguousarray(query[bs]),
                "key_in": np.ascontiguousarray(key_in[bs]),
                "Wq": Wq,
                "Wk": Wk,
                "Wv": Wv,
                "Wo": Wo,
                "bo": bo,
            }
        )
    res = run_bass_kernel_spmd(nc, in_maps, core_ids=list(range(n_cores)))
    out = np.concatenate([res.results[i]["out"] for i in range(n_cores)], axis=0)
    return out.astype(np.float32)


if __name__ == "__main__":
    nc = build_nc(1, 512)
    print("built ok")


# revision 32
# speedup vs baseline: 1.0522x; 1.0178x over previous
"""AssignAttention (hard-routing slot attention) Trainium2 kernel, 8-core data-parallel.

Problem: B=16, N=64 groups, S=4096 tokens, C=768, H=8 heads, HD=96.
  q = query @ Wq.T; k = key @ Wk.T; v = key @ Wv.T (per-head split)
  logits = q @ k.T; hard-argmax over the 64 groups per token -> one-hot
  (softmax and the *SCALE factor are argmax-invariant, so both are skipped);
  attn = onehot / (count + 1); out = (attn @ v per head) @ Wo.T + bo

Sharding: data-parallel over batch B: 16 batches / 8 cores = 2 per core.
No collectives; the host concatenates per-core outputs.

Algorithm per core (validated vs the fp32 reference: rel_l2 ~2e-3, the
residual being argmax flips on near-ties that any reimplementation incurs):
  - The logits are REASSOCIATED: instead of projecting k = key @ Wk.T
    (the dominant 8192x768x768 matmul) and then contracting with q over
    head_dim, we precompute Y[c, (h,n)] = sum_d Wk[d(head h), c] q[n, d]
    (tiny: 768x512 per batch) and compute logits[s, (h,n)] =
    sum_c keyT[c, s] Y[c, (h,n)] -- one 768-contraction matmul produces all
    8 heads' logits, and the k-projection disappears entirely.
  - Precision on the argmax path: fp16 hi/lo split x3 matmuls
    (K@Y ~= Kh@Yh + Kh@Yl + Kl@Yh, fp32 PSUM accumulation, error ~2^-21);
    q-projection the same; Y itself fp32. Head dim is zero-padded 96->128
    because PSUM accumulation groups cannot mix matmul row offsets on HW.
  - argmax via row-max + (x >= max) compare (ties are ~1-ulp rare and only
    perturb one group's mean); counts via an extra N=1 ones-column matmul
    into the same PSUM accumulator; renorm = per-partition reciprocal.
  - keyT (c-major) via PE transpose-mode (fp32, 3 blocks batched per PSUM
    bank, single strided ACT copy out); fp16 halves derived on DVE.
  - v-projection, group-sum, Wo: single-pass fp16 (linear-path error ~5e-4);
    bias via a K=1 fp32 outer-product matmul into the same PSUM group.
  - One accumulation group per 2KB PSUM bank (start zeroes the whole bank).
  - Engine split: PE does matmuls/transposes; DVE does max/is_ge/casts;
    ACT (scalar) does PSUM->SBUF copies; SWDGE does bulk key DMA; HWDGE the
    rest. Measured ~645 us on silicon at 2.4 GHz (~1.36M PE cycles,
    TensorEngine ~88% busy; clock-state dependent).
"""

import sys

if "/opt/trn_rl_repo" not in sys.path:
    sys.path.insert(0, "/opt/trn_rl_repo")

import numpy as np

import concourse.bass as bass
import concourse.mybir as mybir
from concourse import bacc
import concourse.tile as tile
from concourse.masks import make_identity

f32 = mybir.dt.float32
f16 = mybir.dt.float16

C = 768
H = 8
HD = 96
NG = 64  # groups
CT = C // 128  # 6 c-tiles
S_CHUNK = 256


def build_nc(b_sh=2, S=4096):
    nc = bacc.Bacc()

    query_d = nc.declare_dram_parameter("query", [b_sh, NG, C], f32, isOutput=False)
    key_d = nc.declare_dram_parameter("key_in", [b_sh, S, C], f32, isOutput=False)
    wq_d = nc.declare_dram_parameter("Wq", [C, C], f32, isOutput=False)
    wk_d = nc.declare_dram_parameter("Wk", [C, C], f32, isOutput=False)
    wv_d = nc.declare_dram_parameter("Wv", [C, C], f32, isOutput=False)
    wo_d = nc.declare_dram_parameter("Wo", [C, C], f32, isOutput=False)
    bo_d = nc.declare_dram_parameter("bo", [C], f32, isOutput=False)
    out_d = nc.declare_dram_parameter("out", [b_sh, NG, C], f32, isOutput=True)

    n_chunks = S // S_CHUNK
    n_sub = S_CHUNK // 128  # s-subtiles per chunk

    with tile.TileContext(nc) as tc:
        with (
            tc.tile_pool(name="wconst", bufs=1) as wconst,
            tc.tile_pool(name="qpool", bufs=1) as qpool,
            tc.tile_pool(name="kin", bufs=3) as kin,
            tc.tile_pool(name="keyT", bufs=3) as keyTp,
            tc.tile_pool(name="vp", bufs=4) as vp,
            tc.tile_pool(name="ohp", bufs=4) as ohp,
            tc.tile_pool(name="mxp", bufs=3) as mxp,
            tc.tile_pool(name="outp", bufs=1) as outp,
            tc.tile_pool(name="ps_a", bufs=2, space="PSUM") as ps_a,
            tc.tile_pool(name="ps_v", bufs=2, space="PSUM") as ps_v,
            tc.tile_pool(name="ps_tr", bufs=2, space="PSUM") as ps_tr,
            tc.tile_pool(name="ps_gs", bufs=1, space="PSUM") as ps_gs,
        ):
            # ---- constants ----
            ident64_16 = wconst.tile([NG, NG], f16)
            make_identity(nc, ident64_16[:])
            ident64_32 = wconst.tile([NG, NG], f32)
            make_identity(nc, ident64_32[:])
            ident128_16 = wconst.tile([128, 128], f16)
            make_identity(nc, ident128_16[:])
            ident128_32 = wconst.tile([128, 128], f32)
            make_identity(nc, ident128_32[:])
            ones_col = wconst.tile([128, 1], f16)
            nc.vector.memset(ones_col[:], 1.0)
            ones_row = wconst.tile([1, NG], f32)
            nc.vector.memset(ones_row[:], 1.0)
            bo_sb = wconst.tile([1, C], f32)
            nc.sync.dma_start(out=bo_sb[:], in_=bo_d[:].unsqueeze(0))

            def pe_transpose_blocks_f32(src, dst, t):
                """PE-transpose 6 f32 [128,128] blocks src[:, 128u:128u+128]
                into dst[:, u, 128t:128t+128]; 3 blocks per PSUM bank."""
                for g in range(2):
                    trp = ps_tr.tile([128, 3, 128], f32, tag="pstr")
                    for j in range(3):
                        u = 3 * g + j
                        nc.tensor.matmul(
                            trp[:, j, :],
                            src[:, 128 * u : 128 * u + 128],
                            ident128_32[:],
                            is_transpose=True,
                            start=(j == 0),
                            stop=(j == 2),
                        )
                    nc.scalar.copy(
                        out=dst[:, 3 * g : 3 * g + 3, 128 * t : 128 * t + 128],
                        in_=trp[:],
                    )

            def pe_transpose_blocks(src, dst, t, rows=128):
                """PE-transpose 6 f16 [rows,128] blocks src[:, 128u:128u+128]
                (u=0..5) into dst[:, u, 128t:128t+rows] via one batched PSUM
                bank + a single strided ACT copy."""
                ident = ident128_16 if rows == 128 else ident64_16
                trp = ps_tr.tile([128, CT, rows], f16, tag="pstr")
                for u in range(CT):
                    nc.tensor.matmul(
                        trp[:, u, :],
                        src[0:rows, 128 * u : 128 * u + 128],
                        ident[:],
                        is_transpose=True,
                        start=(u == 0),
                        stop=(u == CT - 1),
                    )
                nc.scalar.copy(out=dst[:, :, 128 * t : 128 * t + rows], in_=trp[:])

            # ---- weight prep: transpose to c-major fp16 hi/lo ----
            # wT[p, u, d] = W[d, 128u + p]
            CP = 128 * H  # d-padded width for Wq/Wk (head h at 128h..128h+96)
            wqT_h = wconst.tile([128, CT, CP], f16)
            wqT_l = wconst.tile([128, CT, CP], f16)
            # Wk kept NATURAL fp32 (d-padded rows, c free) for the
            # Y = Wk_h^T q_h precompute; logits = keyT^T @ Y in fp32
            # (~2.1 cyc/row warm measured, and exact precision)
            wk_nat = wconst.tile([128, H, C], f32)
            wvT_h = wconst.tile([128, CT, C], f16)
            woT_h = wconst.tile([128, CT, C], f16)

            # Prefetch the first key chunk so the PE has transpose work
            # while the weight tiles stream in.
            knat_pre = kin.tile([128, n_sub, C], f32, tag="knat")
            nc.gpsimd.dma_start(
                out=knat_pre[:],
                in_=key_d[0, 0:S_CHUNK, :].rearrange("(i p) c -> p i c", p=128),
            )

            # Wq/Wk use the d-padded layout (head h -> cols 128h..128h+96, rest
            # zero) so every logits matmul is a single offset-0 K=128 matmul
            # (mixed-row-offset PSUM accumulation groups fail to load on HW).
            wtmp_ctx = tc.tile_pool(name="wtmp", bufs=4)
            wtmp = wtmp_ctx.__enter__()
            for hd in range(H):
                wnat = wtmp.tile([128, C], f32, tag="wnat")
                nc.vector.memset(wnat[96:128, :], 0.0)
                nc.sync.dma_start(
                    out=wnat[0:HD, :], in_=wq_d[HD * hd : HD * hd + HD, :]
                )
                whi = wtmp.tile([128, C], f16, tag="whi")
                nc.vector.tensor_copy(whi[:], wnat[:])
                pe_transpose_blocks(whi[:], wqT_h[:], hd)
                wlo = wtmp.tile([128, C], f16, tag="wlo")
                nc.vector.tensor_tensor(
                    out=wlo[:], in0=wnat[:], in1=whi[:], op=mybir.AluOpType.subtract
                )
                pe_transpose_blocks(wlo[:], wqT_l[:], hd)
            nc.vector.memset(wk_nat[96:128, :, :], 0.0)
            for hd in range(H):
                nc.sync.dma_start(
                    out=wk_nat[0:HD, hd, :], in_=wk_d[HD * hd : HD * hd + HD, :]
                )
            for w_dram, dst_h in ((wv_d, wvT_h), (wo_d, woT_h)):
                for t in range(CT):
                    wnat = wtmp.tile([128, C], f32, tag="wnat")
                    nc.sync.dma_start(out=wnat[:], in_=w_dram[128 * t : 128 * t + 128, :])
                    whi = wtmp.tile([128, C], f16, tag="whi")
                    nc.vector.tensor_copy(whi[:], wnat[:])
                    pe_transpose_blocks(whi[:], dst_h[:], t)
            wtmp_ctx.__exit__(None, None, None)
            # WvT with a zero column inserted after each head's 96 d-columns:
            # the v-projection then leaves a per-head slot in v16 that a single
            # strided memset turns into the counts ones-column, so each head's
            # group-sum + count is ONE N=97 matmul.
            HD1 = HD + 1
            wvT_p = wconst.tile([128, CT, H * HD1], f16)
            nc.vector.memset(
                wvT_p[:].rearrange("p u (h q) -> p u h q", q=HD1)[:, :, :, HD], 0.0
            )
            for h in range(H):
                nc.vector.tensor_copy(
                    wvT_p[:, :, HD1 * h : HD1 * h + HD],
                    wvT_h[:, :, HD * h : HD * h + HD],
                )

            for b in range(b_sh):
                # ---- Q path ----
                q_nat = qpool.tile([NG, C], f32, tag="qnat")
                nc.sync.dma_start(out=q_nat[:], in_=query_d[b])
                qh_nat = qpool.tile([NG, C], f16, tag="qhnat")
                ql_nat = qpool.tile([NG, C], f16, tag="qlnat")
                nc.vector.tensor_copy(qh_nat[:], q_nat[:])
                nc.vector.tensor_tensor(
                    out=ql_nat[:], in0=q_nat[:], in1=qh_nat[:], op=mybir.AluOpType.subtract
                )
                # queryT (c-major) fp16 halves via DMA xbar ([64,128] blocks)
                qTq_h = qpool.tile([128, CT, NG], f16, tag="qTqh")
                qTq_l = qpool.tile([128, CT, NG], f16, tag="qTql")
                for qsrc, dst in ((qh_nat, qTq_h), (ql_nat, qTq_l)):
                    pe_transpose_blocks(qsrc, dst[:].unsqueeze(3).rearrange("p u n o -> p u (n o)"), 0, rows=NG)
                # q projection (natural layout, M=64), d-padded: q_pad [64, 1024]
                q_sb = qpool.tile([NG, CP], f32, tag="qsb")
                for half in range(2):
                    nsl = slice(512 * half, 512 * half + 512)
                    qp = ps_a.tile([NG, 512], f32, tag="psa")
                    first = True
                    for u in range(CT):
                        for lhsT, rhs in (
                            (qTq_h, wqT_h),
                            (qTq_h, wqT_l),
                            (qTq_l, wqT_h),
                        ):
                            nc.tensor.matmul(
                                qp[:],
                                lhsT[:, u, :],
                                rhs[:, u, nsl],
                                start=first,
                                stop=(u == CT - 1 and lhsT is qTq_l),
                            )
                            first = False
                    nc.scalar.copy(out=q_sb[:, nsl], in_=qp[:])
                # qT (padded d-major, per head) fp32 via PE transpose
                qT = qpool.tile([128, H, NG], f32, tag="qT")
                for hd in range(H):
                    trq2 = ps_a.tile([128, NG], f32, tag="psa")
                    nc.tensor.matmul(
                        trq2[:],
                        q_sb[:, 128 * hd : 128 * hd + 128],
                        ident64_32[:],
                        is_transpose=True,
                        start=True,
                        stop=True,
                    )
                    nc.scalar.copy(out=qT[:, hd, :], in_=trq2[:])
                # Y_all[c, 64h+n] = sum_d Wk[d(head h), c] * q[n, d], fp32,
                # then fp16 hi/lo split. logits = keyT^T @ Y_all (split x3).
                Y_h = qpool.tile([128, CT, 8 * NG], f16, tag="Yh")
                Y_l = qpool.tile([128, CT, 8 * NG], f16, tag="Yl")
                for u_c in range(CT):
                    yp = ps_a.tile([128, 8 * NG], f32, tag="psa")
                    csl = slice(128 * u_c, 128 * u_c + 128)
                    for hd in range(H):
                        nc.tensor.matmul(
                            yp[:, NG * hd : NG * hd + NG],
                            wk_nat[:, hd, csl],
                            qT[:, hd, :],
                            start=(hd == 0),
                            stop=(hd == H - 1),
                        )
                    nc.vector.tensor_copy(Y_h[:, u_c, :], yp[:])
                    nc.vector.tensor_tensor(
                        out=Y_l[:, u_c, :], in0=yp[:], in1=Y_h[:, u_c, :],
                        op=mybir.AluOpType.subtract,
                    )

                # ---- group-sum accumulator for this b ----
                gs = ps_gs.tile([NG, 8 * 128], f32, tag="gs")

                for chunk in range(n_chunks):
                    s0 = chunk * S_CHUNK
                    # load key chunk (chunk 0 of b 0 was prefetched)
                    if b == 0 and chunk == 0:
                        knat = knat_pre
                    else:
                        knat = kin.tile([128, n_sub, C], f32, tag="knat")
                        nc.gpsimd.dma_start(
                            out=knat[:],
                            in_=key_d[b, s0 : s0 + S_CHUNK, :].rearrange(
                                "(i p) c -> p i c", p=128
                            ),
                        )
                    # keyT via fp32 PE transposes, then fp16 hi/lo split
                    # (hi also feeds the v-projection)
                    keyT = keyTp.tile([128, CT, S_CHUNK], f32, tag="keyT")
                    for i in range(n_sub):
                        pe_transpose_blocks_f32(knat[:, i, :], keyT[:], i)
                    kTh = keyTp.tile([128, CT, S_CHUNK], f16, tag="kTh")
                    kTl = keyTp.tile([128, CT, S_CHUNK], f16, tag="kTl")
                    nc.vector.tensor_copy(kTh[:], keyT[:])
                    nc.vector.tensor_tensor(
                        out=kTl[:], in0=keyT[:], in1=kTh[:], op=mybir.AluOpType.subtract
                    )

                    for i in range(n_sub):
                        ssl = slice(128 * i, 128 * i + 128)
                        # v projection (natural [s, d]) fp16 single pass;
                        # two single-bank PSUM tiles so copies pipeline
                        vpsA = ps_v.tile([128, 388], f32, tag="vps")
                        vpsB = ps_v.tile([128, 388], f32, tag="vps")
                        for u_c in range(CT):
                            nc.tensor.matmul(
                                vpsA[:],
                                kTh[:, u_c, ssl],
                                wvT_p[:, u_c, 0:388],
                                start=(u_c == 0),
                                stop=(u_c == CT - 1),
                            )
                            nc.tensor.matmul(
                                vpsB[:],
                                kTh[:, u_c, ssl],
                                wvT_p[:, u_c, 388:776],
                                start=(u_c == 0),
                                stop=(u_c == CT - 1),
                            )
                        v16 = vp.tile([128, H * HD1], f16, tag="v16")
                        nc.scalar.copy(out=v16[:, 0:388], in_=vpsA[:])
                        nc.scalar.copy(out=v16[:, 388:776], in_=vpsB[:])
                        nc.vector.memset(
                            v16[:].rearrange("p (h q) -> p h q", q=HD1)[:, :, HD], 1.0
                        )

                        # logits for all 8 heads at once: lg[s, 64h+n] =
                        # sum_c keyT[c, s] Y_all[c, 64h+n], fp16 split x3.
                        # One accumulation group per PSUM bank: start only on
                        # the first matmul (zeroes the 2KB region), stop last.
                        lg = ps_a.tile([128, 8 * NG], f32, tag="psa")
                        first = True
                        for u_c in range(CT):
                            for kt, yt in ((kTh, Y_h), (kTh, Y_l), (kTl, Y_h)):
                                nc.tensor.matmul(
                                    lg[:],
                                    kt[:, u_c, ssl],
                                    yt[:, u_c, :],
                                    start=first,
                                    stop=(u_c == CT - 1 and kt is kTl),
                                )
                                first = False
                        # argmax -> one-hot via (x >= rowmax), fp16
                        mx = mxp.tile([128, H], f32, tag="mx")
                        lg3 = lg[:].rearrange("p (h n) -> p h n", h=H)
                        nc.vector.tensor_reduce(
                            out=mx[:],
                            in_=lg3,
                            axis=mybir.AxisListType.X,
                            op=mybir.AluOpType.max,
                        )
                        oh = ohp.tile([128, H * NG], f16, tag="oh")
                        nc.vector.tensor_tensor(
                            out=oh[:].rearrange("p (h n) -> p h n", h=H),
                            in0=lg3,
                            in1=mx[:].unsqueeze(2).to_broadcast((128, H, NG)),
                            op=mybir.AluOpType.is_ge,
                        )

                        # group sums + counts (fp16 matmuls, fp32 accum)
                        # gs spans 2 PSUM banks (heads 0-3, heads 4-7): one
                        # start per bank (zeroes the 2KB region), one stop
                        last = chunk == n_chunks - 1 and i == n_sub - 1
                        first = chunk == 0 and i == 0
                        for h in range(H):
                            nc.tensor.matmul(
                                gs[:, 128 * h : 128 * h + HD1],
                                oh[:, NG * h : NG * h + NG],
                                v16[:, HD1 * h : HD1 * h + HD1],
                                start=(first and h in (0, 4)),
                                stop=(last and h in (3, 7)),
                            )

                # ---- finalize b: divide by (count+1), transpose, Wo, bias ----
                cnt = outp.tile([NG, H], f32, tag="cnt")
                nc.vector.tensor_scalar(
                    out=cnt[:],
                    in0=gs[:].rearrange("p (h q) -> p h q", q=128)[:, :, HD],
                    scalar1=1.0,
                    scalar2=None,
                    op0=mybir.AluOpType.add,
                )
                rec = outp.tile([NG, H], f32, tag="rec")
                nc.vector.reciprocal(rec[:], cnt[:])
                attn16 = outp.tile([NG, C], f16, tag="attn16")
                for h in range(H):
                    nc.vector.tensor_scalar(
                        out=attn16[:, HD * h : HD * h + HD],
                        in0=gs[:, 128 * h : 128 * h + HD],
                        scalar1=rec[:, h : h + 1],
                        scalar2=None,
                        op0=mybir.AluOpType.mult,
                    )
                attnT = outp.tile([128, CT, NG], f16, tag="attnT")
                pe_transpose_blocks(attn16, attnT[:].unsqueeze(3).rearrange("p u n o -> p u (n o)"), 0, rows=NG)

                out_sb = outp.tile([NG, C], f32, tag="outsb")
                for half in range(2):
                    nsl = slice(384 * half, 384 * half + 384)
                    op = ps_a.tile([NG, 384], f32, tag="psa")
                    for u_c in range(CT):
                        nc.tensor.matmul(
                            op[:],
                            attnT[:, u_c, :],
                            woT_h[:, u_c, nsl],
                            start=(u_c == 0),
                            stop=False,
                        )
                    nc.tensor.matmul(
                        op[:], ones_row[:], bo_sb[:, nsl], start=False, stop=True
                    )
                    nc.scalar.copy(out=out_sb[:, nsl], in_=op[:])
                nc.gpsimd.dma_start(out=out_d[b], in_=out_sb[:])

    nc.finalize()
    return nc


_NC_CACHE = {}


def _get_nc(b_sh, S):
    key = (b_sh, S)
    if key not in _NC_CACHE:
        _NC_CACHE[key] = build_nc(b_sh, S)
    return _NC_CACHE[key]


def kernel(query, key_in, Wq, Wk, Wv, Wo, bo):
    from concourse.bass_utils import run_bass_kernel_spmd

    query = np.ascontiguousarray(np.asarray(query, dtype=np.float32))
    key_in = np.ascontiguousarray(np.asarray(key_in, dtype=np.float32))
    Wq = np.ascontiguousarray(np.asarray(Wq, dtype=np.float32))
    Wk = np.ascontiguousarray(np.asarray(Wk, dtype=np.float32))
    Wv = np.ascontiguousarray(np.asarray(Wv, dtype=np.float32))
    Wo = np.ascontiguousarray(np.asarray(Wo, dtype=np.float32))
    bo = np.ascontiguousarray(np.asarray(bo, dtype=np.float32))

    B, _, _ = query.shape
    S = key_in.shape[1]
    n_cores = 8
    b_sh = B // n_cores
    nc = _get_nc(b_sh, S)

    in_maps = []
    for i in range(n_cores):
        bs = slice(i * b_sh, (i + 1) * b_sh)
        in_maps.append(
            {
                "query": np.ascontiguousarray(query[bs]),
                "key_in": np.ascontiguousarray(key_in[bs]),
                "Wq": Wq,
                "Wk": Wk,
                "Wv": Wv,
                "Wo": Wo,
                "bo": bo,
            }
        )
    res = run_bass_kernel_spmd(nc, in_maps, core_ids=list(range(n_cores)))
    out = np.concatenate([res.results[i]["out"] for i in range(n_cores)], axis=0)
    return out.astype(np.float32)


if __name__ == "__main__":
    nc = build_nc(1, 512)
    print("built ok")


# revision 33
# speedup vs baseline: 1.0571x; 1.0047x over previous
"""AssignAttention (hard-routing slot attention) Trainium2 kernel, 8-core data-parallel.

Problem: B=16, N=64 groups, S=4096 tokens, C=768, H=8 heads, HD=96.
  q = query @ Wq.T; k = key @ Wk.T; v = key @ Wv.T (per-head split)
  logits = q @ k.T; hard-argmax over the 64 groups per token -> one-hot
  (softmax and the *SCALE factor are argmax-invariant, so both are skipped);
  attn = onehot / (count + 1); out = (attn @ v per head) @ Wo.T + bo

Sharding: data-parallel over batch B: 16 batches / 8 cores = 2 per core.
No collectives; the host concatenates per-core outputs.

Algorithm per core (validated vs the fp32 reference: rel_l2 ~2e-3, the
residual being argmax flips on near-ties that any reimplementation incurs):
  - The logits are REASSOCIATED: instead of projecting k = key @ Wk.T
    (the dominant 8192x768x768 matmul) and then contracting with q over
    head_dim, we precompute Y[c, (h,n)] = sum_d Wk[d(head h), c] q[n, d]
    (tiny: 768x512 per batch) and compute logits[s, (h,n)] =
    sum_c keyT[c, s] Y[c, (h,n)] -- one 768-contraction matmul produces all
    8 heads' logits, and the k-projection disappears entirely.
  - Precision on the argmax path: fp16 hi/lo split x3 matmuls
    (K@Y ~= Kh@Yh + Kh@Yl + Kl@Yh, fp32 PSUM accumulation, error ~2^-21);
    q-projection the same; Y itself fp32. Head dim is zero-padded 96->128
    because PSUM accumulation groups cannot mix matmul row offsets on HW.
  - argmax via row-max + (x >= max) compare (ties are ~1-ulp rare and only
    perturb one group's mean); counts via an extra N=1 ones-column matmul
    into the same PSUM accumulator; renorm = per-partition reciprocal.
  - keyT (c-major) via PE transpose-mode (fp32, 3 blocks batched per PSUM
    bank, single strided ACT copy out); fp16 halves derived on DVE.
  - v-projection, group-sum, Wo: single-pass fp16 (linear-path error ~5e-4);
    bias via a K=1 fp32 outer-product matmul into the same PSUM group.
  - One accumulation group per 2KB PSUM bank (start zeroes the whole bank).
  - Engine split: PE does matmuls/transposes; DVE does max/is_ge/casts;
    ACT (scalar) does PSUM->SBUF copies; SWDGE does bulk key DMA; HWDGE the
    rest. Measured ~645 us on silicon at 2.4 GHz (~1.36M PE cycles,
    TensorEngine ~88% busy; clock-state dependent).
"""

import sys

if "/opt/trn_rl_repo" not in sys.path:
    sys.path.insert(0, "/opt/trn_rl_repo")

import numpy as np

import concourse.bass as bass
import concourse.mybir as mybir
from concourse import bacc
import concourse.tile as tile
from concourse.masks import make_identity

f32 = mybir.dt.float32
f16 = mybir.dt.float16

C = 768
H = 8
HD = 96
NG = 64  # groups
CT = C // 128  # 6 c-tiles
S_CHUNK = 256


def build_nc(b_sh=2, S=4096):
    nc = bacc.Bacc()

    query_d = nc.declare_dram_parameter("query", [b_sh, NG, C], f32, isOutput=False)
    key_d = nc.declare_dram_parameter("key_in", [b_sh, S, C], f32, isOutput=False)
    wq_d = nc.declare_dram_parameter("Wq", [C, C], f32, isOutput=False)
    wk_d = nc.declare_dram_parameter("Wk", [C, C], f32, isOutput=False)
    wv_d = nc.declare_dram_parameter("Wv", [C, C], f32, isOutput=False)
    wo_d = nc.declare_dram_parameter("Wo", [C, C], f32, isOutput=False)
    bo_d = nc.declare_dram_parameter("bo", [C], f32, isOutput=False)
    out_d = nc.declare_dram_parameter("out", [b_sh, NG, C], f32, isOutput=True)

    n_chunks = S // S_CHUNK
    n_sub = S_CHUNK // 128  # s-subtiles per chunk

    with tile.TileContext(nc) as tc:
        with (
            tc.tile_pool(name="wconst", bufs=1) as wconst,
            tc.tile_pool(name="qpool", bufs=1) as qpool,
            tc.tile_pool(name="kin", bufs=3) as kin,
            tc.tile_pool(name="keyT", bufs=3) as keyTp,
            tc.tile_pool(name="vp", bufs=4) as vp,
            tc.tile_pool(name="ohp", bufs=4) as ohp,
            tc.tile_pool(name="mxp", bufs=3) as mxp,
            tc.tile_pool(name="outp", bufs=1) as outp,
            tc.tile_pool(name="ps_a", bufs=2, space="PSUM") as ps_a,
            tc.tile_pool(name="ps_v", bufs=2, space="PSUM") as ps_v,
            tc.tile_pool(name="ps_tr", bufs=2, space="PSUM") as ps_tr,
            tc.tile_pool(name="ps_gs", bufs=1, space="PSUM") as ps_gs,
        ):
            # ---- constants ----
            ident64_16 = wconst.tile([NG, NG], f16)
            make_identity(nc, ident64_16[:])
            ident64_32 = wconst.tile([NG, NG], f32)
            make_identity(nc, ident64_32[:])
            ident128_16 = wconst.tile([128, 128], f16)
            make_identity(nc, ident128_16[:])
            ident128_32 = wconst.tile([128, 128], f32)
            make_identity(nc, ident128_32[:])
            ones_row = wconst.tile([1, NG], f32)
            nc.vector.memset(ones_row[:], 1.0)
            bo_sb = wconst.tile([1, C], f32)
            nc.sync.dma_start(out=bo_sb[:], in_=bo_d[:].unsqueeze(0))

            def pe_transpose_blocks_f32(src, dst, t):
                """PE-transpose 6 f32 [128,128] blocks src[:, 128u:128u+128]
                into dst[:, u, 128t:128t+128]; 3 blocks per PSUM bank."""
                for g in range(2):
                    trp = ps_tr.tile([128, 3, 128], f32, tag="pstr")
                    for j in range(3):
                        u = 3 * g + j
                        nc.tensor.matmul(
                            trp[:, j, :],
                            src[:, 128 * u : 128 * u + 128],
                            ident128_32[:],
                            is_transpose=True,
                            start=(j == 0),
                            stop=(j == 2),
                        )
                    nc.scalar.copy(
                        out=dst[:, 3 * g : 3 * g + 3, 128 * t : 128 * t + 128],
                        in_=trp[:],
                    )

            def pe_transpose_blocks(src, dst, t, rows=128):
                """PE-transpose 6 f16 [rows,128] blocks src[:, 128u:128u+128]
                (u=0..5) into dst[:, u, 128t:128t+rows] via one batched PSUM
                bank + a single strided ACT copy."""
                ident = ident128_16 if rows == 128 else ident64_16
                trp = ps_tr.tile([128, CT, rows], f16, tag="pstr")
                for u in range(CT):
                    nc.tensor.matmul(
                        trp[:, u, :],
                        src[0:rows, 128 * u : 128 * u + 128],
                        ident[:],
                        is_transpose=True,
                        start=(u == 0),
                        stop=(u == CT - 1),
                    )
                nc.scalar.copy(out=dst[:, :, 128 * t : 128 * t + rows], in_=trp[:])

            # ---- weight prep: transpose to c-major fp16 hi/lo ----
            # wT[p, u, d] = W[d, 128u + p]
            CP = 128 * H  # d-padded width for Wq/Wk (head h at 128h..128h+96)
            wqT_h = wconst.tile([128, CT, CP], f16)
            wqT_l = wconst.tile([128, CT, CP], f16)
            # Wk kept NATURAL fp32 (d-padded rows, c free) for the
            # Y = Wk_h^T q_h precompute; logits = keyT^T @ Y in fp32
            # (~2.1 cyc/row warm measured, and exact precision)
            wk_nat = wconst.tile([128, H, C], f32)
            wvT_h = wconst.tile([128, CT, C], f16)
            woT_h = wconst.tile([128, CT, C], f16)

            # Prefetch the first key chunk so the PE has transpose work
            # while the weight tiles stream in.
            knat_pre = kin.tile([128, n_sub, C], f32, tag="knat")
            nc.gpsimd.dma_start(
                out=knat_pre[:],
                in_=key_d[0, 0:S_CHUNK, :].rearrange("(i p) c -> p i c", p=128),
            )

            # Wq/Wk use the d-padded layout (head h -> cols 128h..128h+96, rest
            # zero) so every logits matmul is a single offset-0 K=128 matmul
            # (mixed-row-offset PSUM accumulation groups fail to load on HW).
            wtmp_ctx = tc.tile_pool(name="wtmp", bufs=4)
            wtmp = wtmp_ctx.__enter__()
            for hd in range(H):
                wnat = wtmp.tile([128, C], f32, tag="wnat")
                nc.vector.memset(wnat[96:128, :], 0.0)
                nc.sync.dma_start(
                    out=wnat[0:HD, :], in_=wq_d[HD * hd : HD * hd + HD, :]
                )
                whi = wtmp.tile([128, C], f16, tag="whi")
                nc.vector.tensor_copy(whi[:], wnat[:])
                pe_transpose_blocks(whi[:], wqT_h[:], hd)
                wlo = wtmp.tile([128, C], f16, tag="wlo")
                nc.vector.tensor_tensor(
                    out=wlo[:], in0=wnat[:], in1=whi[:], op=mybir.AluOpType.subtract
                )
                pe_transpose_blocks(wlo[:], wqT_l[:], hd)
            nc.vector.memset(wk_nat[96:128, :, :], 0.0)
            for hd in range(H):
                nc.sync.dma_start(
                    out=wk_nat[0:HD, hd, :], in_=wk_d[HD * hd : HD * hd + HD, :]
                )
            for w_dram, dst_h in ((wv_d, wvT_h), (wo_d, woT_h)):
                for t in range(CT):
                    wnat = wtmp.tile([128, C], f32, tag="wnat")
                    nc.sync.dma_start(out=wnat[:], in_=w_dram[128 * t : 128 * t + 128, :])
                    whi = wtmp.tile([128, C], f16, tag="whi")
                    nc.vector.tensor_copy(whi[:], wnat[:])
                    pe_transpose_blocks(whi[:], dst_h[:], t)
            wtmp_ctx.__exit__(None, None, None)
            # WvT with a zero column inserted after each head's 96 d-columns:
            # the v-projection then leaves a per-head slot in v16 that a single
            # strided memset turns into the counts ones-column, so each head's
            # group-sum + count is ONE N=97 matmul.
            HD1 = HD + 1
            wvT_p = wconst.tile([128, CT, H * HD1], f16)
            nc.vector.memset(
                wvT_p[:].rearrange("p u (h q) -> p u h q", q=HD1)[:, :, :, HD], 0.0
            )
            for h in range(H):
                nc.vector.tensor_copy(
                    wvT_p[:, :, HD1 * h : HD1 * h + HD],
                    wvT_h[:, :, HD * h : HD * h + HD],
                )

            for b in range(b_sh):
                # ---- Q path ----
                q_nat = qpool.tile([NG, C], f32, tag="qnat")
                nc.sync.dma_start(out=q_nat[:], in_=query_d[b])
                qh_nat = qpool.tile([NG, C], f16, tag="qhnat")
                ql_nat = qpool.tile([NG, C], f16, tag="qlnat")
                nc.vector.tensor_copy(qh_nat[:], q_nat[:])
                nc.vector.tensor_tensor(
                    out=ql_nat[:], in0=q_nat[:], in1=qh_nat[:], op=mybir.AluOpType.subtract
                )
                # queryT (c-major) fp16 halves via DMA xbar ([64,128] blocks)
                qTq_h = qpool.tile([128, CT, NG], f16, tag="qTqh")
                qTq_l = qpool.tile([128, CT, NG], f16, tag="qTql")
                for qsrc, dst in ((qh_nat, qTq_h), (ql_nat, qTq_l)):
                    pe_transpose_blocks(qsrc, dst[:].unsqueeze(3).rearrange("p u n o -> p u (n o)"), 0, rows=NG)
                # q projection (natural layout, M=64), d-padded: q_pad [64, 1024]
                q_sb = qpool.tile([NG, CP], f32, tag="qsb")
                for half in range(2):
                    nsl = slice(512 * half, 512 * half + 512)
                    qp = ps_a.tile([NG, 512], f32, tag="psa")
                    first = True
                    for u in range(CT):
                        for lhsT, rhs in (
                            (qTq_h, wqT_h),
                            (qTq_h, wqT_l),
                            (qTq_l, wqT_h),
                        ):
                            nc.tensor.matmul(
                                qp[:],
                                lhsT[:, u, :],
                                rhs[:, u, nsl],
                                start=first,
                                stop=(u == CT - 1 and lhsT is qTq_l),
                            )
                            first = False
                    nc.scalar.copy(out=q_sb[:, nsl], in_=qp[:])
                # qT (padded d-major, per head) fp32 via PE transpose
                qT = qpool.tile([128, H, NG], f32, tag="qT")
                for hd in range(H):
                    trq2 = ps_a.tile([128, NG], f32, tag="psa")
                    nc.tensor.matmul(
                        trq2[:],
                        q_sb[:, 128 * hd : 128 * hd + 128],
                        ident64_32[:],
                        is_transpose=True,
                        start=True,
                        stop=True,
                    )
                    nc.scalar.copy(out=qT[:, hd, :], in_=trq2[:])
                # Y_all[c, 64h+n] = sum_d Wk[d(head h), c] * q[n, d], fp32,
                # then fp16 hi/lo split. logits = keyT^T @ Y_all (split x3).
                Y_h = qpool.tile([128, CT, 8 * NG], f16, tag="Yh")
                Y_l = qpool.tile([128, CT, 8 * NG], f16, tag="Yl")
                for u_c in range(CT):
                    yp = ps_a.tile([128, 8 * NG], f32, tag="psa")
                    csl = slice(128 * u_c, 128 * u_c + 128)
                    for hd in range(H):
                        nc.tensor.matmul(
                            yp[:, NG * hd : NG * hd + NG],
                            wk_nat[:, hd, csl],
                            qT[:, hd, :],
                            start=(hd == 0),
                            stop=(hd == H - 1),
                        )
                    nc.vector.tensor_copy(Y_h[:, u_c, :], yp[:])
                    nc.vector.tensor_tensor(
                        out=Y_l[:, u_c, :], in0=yp[:], in1=Y_h[:, u_c, :],
                        op=mybir.AluOpType.subtract,
                    )

                # ---- group-sum accumulator for this b ----
                gs = ps_gs.tile([NG, 8 * 128], f32, tag="gs")

                for chunk in range(n_chunks):
                    s0 = chunk * S_CHUNK
                    # load key chunk (chunk 0 of b 0 was prefetched)
                    if b == 0 and chunk == 0:
                        knat = knat_pre
                    else:
                        knat = kin.tile([128, n_sub, C], f32, tag="knat")
                        nc.gpsimd.dma_start(
                            out=knat[:],
                            in_=key_d[b, s0 : s0 + S_CHUNK, :].rearrange(
                                "(i p) c -> p i c", p=128
                            ),
                        )
                    # keyT via fp32 PE transposes, then fp16 hi/lo split
                    # (hi also feeds the v-projection)
                    keyT = keyTp.tile([128, CT, S_CHUNK], f32, tag="keyT")
                    for i in range(n_sub):
                        pe_transpose_blocks_f32(knat[:, i, :], keyT[:], i)
                    kTh = keyTp.tile([128, CT, S_CHUNK], f16, tag="kTh")
                    kTl = keyTp.tile([128, CT, S_CHUNK], f16, tag="kTl")
                    nc.vector.tensor_copy(kTh[:], keyT[:])
                    nc.vector.tensor_tensor(
                        out=kTl[:], in0=keyT[:], in1=kTh[:], op=mybir.AluOpType.subtract
                    )

                    for i in range(n_sub):
                        ssl = slice(128 * i, 128 * i + 128)
                        # v projection (natural [s, d]) fp16 single pass;
                        # two single-bank PSUM tiles so copies pipeline
                        vpsA = ps_v.tile([128, 388], f32, tag="vps")
                        vpsB = ps_v.tile([128, 388], f32, tag="vps")
                        for u_c in range(CT):
                            nc.tensor.matmul(
                                vpsA[:],
                                kTh[:, u_c, ssl],
                                wvT_p[:, u_c, 0:388],
                                start=(u_c == 0),
                                stop=(u_c == CT - 1),
                            )
                            nc.tensor.matmul(
                                vpsB[:],
                                kTh[:, u_c, ssl],
                                wvT_p[:, u_c, 388:776],
                                start=(u_c == 0),
                                stop=(u_c == CT - 1),
                            )
                        v16 = vp.tile([128, H * HD1], f16, tag="v16")
                        nc.scalar.copy(out=v16[:, 0:388], in_=vpsA[:])
                        nc.scalar.copy(out=v16[:, 388:776], in_=vpsB[:])
                        nc.vector.memset(
                            v16[:].rearrange("p (h q) -> p h q", q=HD1)[:, :, HD], 1.0
                        )

                        # logits for all 8 heads at once: lg[s, 64h+n] =
                        # sum_c keyT[c, s] Y_all[c, 64h+n], fp16 split x3.
                        # One accumulation group per PSUM bank: start only on
                        # the first matmul (zeroes the 2KB region), stop last.
                        lg = ps_a.tile([128, 8 * NG], f32, tag="psa")
                        first = True
                        for u_c in range(CT):
                            for kt, yt in ((kTh, Y_h), (kTh, Y_l), (kTl, Y_h)):
                                nc.tensor.matmul(
                                    lg[:],
                                    kt[:, u_c, ssl],
                                    yt[:, u_c, :],
                                    start=first,
                                    stop=(u_c == CT - 1 and kt is kTl),
                                )
                                first = False
                        # argmax -> one-hot via (x >= rowmax), fp16
                        mx = mxp.tile([128, H], f32, tag="mx")
                        lg3 = lg[:].rearrange("p (h n) -> p h n", h=H)
                        nc.vector.tensor_reduce(
                            out=mx[:],
                            in_=lg3,
                            axis=mybir.AxisListType.X,
                            op=mybir.AluOpType.max,
                        )
                        oh = ohp.tile([128, H * NG], f16, tag="oh")
                        nc.vector.tensor_tensor(
                            out=oh[:].rearrange("p (h n) -> p h n", h=H),
                            in0=lg3,
                            in1=mx[:].unsqueeze(2).to_broadcast((128, H, NG)),
                            op=mybir.AluOpType.is_ge,
                        )

                        # group sums + counts (fp16 matmuls, fp32 accum)
                        # gs spans 2 PSUM banks (heads 0-3, heads 4-7): one
                        # start per bank (zeroes the 2KB region), one stop
                        last = chunk == n_chunks - 1 and i == n_sub - 1
                        first = chunk == 0 and i == 0
                        for h in range(H):
                            nc.tensor.matmul(
                                gs[:, 128 * h : 128 * h + HD1],
                                oh[:, NG * h : NG * h + NG],
                                v16[:, HD1 * h : HD1 * h + HD1],
                                start=(first and h in (0, 4)),
                                stop=(last and h in (3, 7)),
                            )

                # ---- finalize b: divide by (count+1), transpose, Wo, bias ----
                cnt = outp.tile([NG, H], f32, tag="cnt")
                nc.vector.tensor_scalar(
                    out=cnt[:],
                    in0=gs[:].rearrange("p (h q) -> p h q", q=128)[:, :, HD],
                    scalar1=1.0,
                    scalar2=None,
                    op0=mybir.AluOpType.add,
                )
                rec = outp.tile([NG, H], f32, tag="rec")
                nc.vector.reciprocal(rec[:], cnt[:])
                attn16 = outp.tile([NG, C], f16, tag="attn16")
                for h in range(H):
                    nc.vector.tensor_scalar(
                        out=attn16[:, HD * h : HD * h + HD],
                        in0=gs[:, 128 * h : 128 * h + HD],
                        scalar1=rec[:, h : h + 1],
                        scalar2=None,
                        op0=mybir.AluOpType.mult,
                    )
                attnT = outp.tile([128, CT, NG], f16, tag="attnT")
                pe_transpose_blocks(attn16, attnT[:].unsqueeze(3).rearrange("p u n o -> p u (n o)"), 0, rows=NG)

                out_sb = outp.tile([NG, C], f32, tag="outsb")
                for half in range(2):
                    nsl = slice(384 * half, 384 * half + 384)
                    op = ps_a.tile([NG, 384], f32, tag="psa")
                    for u_c in range(CT):
                        nc.tensor.matmul(
                            op[:],
                            attnT[:, u_c, :],
                            woT_h[:, u_c, nsl],
                            start=(u_c == 0),
                            stop=False,
                        )
                    nc.tensor.matmul(
                        op[:], ones_row[:], bo_sb[:, nsl], start=False, stop=True
                    )
                    nc.scalar.copy(out=out_sb[:, nsl], in_=op[:])
                nc.gpsimd.dma_start(out=out_d[b], in_=out_sb[:])

    nc.finalize()
    return nc


_NC_CACHE = {}


def _get_nc(b_sh, S):
    key = (b_sh, S)
    if key not in _NC_CACHE:
        _NC_CACHE[key] = build_nc(b_sh, S)
    return _NC_CACHE[key]


def kernel(query, key_in, Wq, Wk, Wv, Wo, bo):
    from concourse.bass_utils import run_bass_kernel_spmd

    query = np.ascontiguousarray(np.asarray(query, dtype=np.float32))
    key_in = np.ascontiguousarray(np.asarray(key_in, dtype=np.float32))
    Wq = np.ascontiguousarray(np.asarray(Wq, dtype=np.float32))
    Wk = np.ascontiguousarray(np.asarray(Wk, dtype=np.float32))
    Wv = np.ascontiguousarray(np.asarray(Wv, dtype=np.float32))
    Wo = np.ascontiguousarray(np.asarray(Wo, dtype=np.float32))
    bo = np.ascontiguousarray(np.asarray(bo, dtype=np.float32))

    B, _, _ = query.shape
    S = key_in.shape[1]
    n_cores = 8
    b_sh = B // n_cores
    nc = _get_nc(b_sh, S)

    in_maps = []
    for i in range(n_cores):
        bs = slice(i * b_sh, (i + 1) * b_sh)
        in_maps.append(
            {
                "query": np.ascontiguousarray(query[bs]),
                "key_in": np.ascontiguousarray(key_in[bs]),
                "Wq": Wq,
                "Wk": Wk,
                "Wv": Wv,
                "Wo": Wo,
                "bo": bo,
            }
        )
    res = run_bass_kernel_spmd(nc, in_maps, core_ids=list(range(n_cores)))
    out = np.concatenate([res.results[i]["out"] for i in range(n_cores)], axis=0)
    return out.astype(np.float32)


if __name__ == "__main__":
    nc = build_nc(1, 512)
    print("built ok")


# revision 36
# speedup vs baseline: 1.2474x; 1.1800x over previous
"""AssignAttention (hard-routing slot attention) Trainium2 kernel, 8-core data-parallel.

Problem: B=16, N=64 groups, S=4096 tokens, C=768, H=8 heads, HD=96.
  q = query @ Wq.T; k = key @ Wk.T; v = key @ Wv.T (per-head split)
  logits = q @ k.T; hard-argmax over the 64 groups per token -> one-hot
  (softmax and the *SCALE factor are argmax-invariant, so both are skipped);
  attn = onehot / (count + 1); out = (attn @ v per head) @ Wo.T + bo

Sharding: data-parallel over batch B: 16 batches / 8 cores = 2 per core.
No collectives; the host concatenates per-core outputs.

Algorithm per core (validated vs the fp32 reference: rel_l2 ~2e-3, the
residual being argmax flips on near-ties that any reimplementation incurs):
  - The logits are REASSOCIATED: instead of projecting k = key @ Wk.T
    (the dominant 8192x768x768 matmul) and then contracting with q over
    head_dim, we precompute Y[c, (h,n)] = sum_d Wk[d(head h), c] q[n, d]
    (tiny: 768x512 per batch) and compute logits[s, (h,n)] =
    sum_c keyT[c, s] Y[c, (h,n)] -- one 768-contraction matmul produces all
    8 heads' logits, and the k-projection disappears entirely.
  - Precision on the argmax path: fp16 hi/lo split x3 matmuls
    (K@Y ~= Kh@Yh + Kh@Yl + Kl@Yh, fp32 PSUM accumulation, error ~2^-21);
    q-projection the same; Y itself fp32. Head dim is zero-padded 96->128
    because PSUM accumulation groups cannot mix matmul row offsets on HW.
  - argmax via row-max + (x >= max) compare (ties are ~1-ulp rare and only
    perturb one group's mean); counts via an extra N=1 ones-column matmul
    into the same PSUM accumulator; renorm = per-partition reciprocal.
  - keyT (c-major) via PE transpose-mode (fp32, 3 blocks batched per PSUM
    bank, single strided ACT copy out); fp16 halves derived on DVE.
  - v-projection, group-sum, Wo: single-pass fp16 (linear-path error ~5e-4);
    bias via a K=1 fp32 outer-product matmul into the same PSUM group.
  - One accumulation group per 2KB PSUM bank (start zeroes the whole bank).
  - Engine split: PE does matmuls/transposes; DVE does max/is_ge/casts;
    ACT (scalar) does PSUM->SBUF copies; SWDGE does bulk key DMA; HWDGE the
    rest. Measured ~645 us on silicon at 2.4 GHz (~1.36M PE cycles,
    TensorEngine ~88% busy; clock-state dependent).
"""

import sys

if "/opt/trn_rl_repo" not in sys.path:
    sys.path.insert(0, "/opt/trn_rl_repo")

import numpy as np

import concourse.bass as bass
import concourse.mybir as mybir
from concourse import bacc
import concourse.tile as tile
from concourse.masks import make_identity

f32 = mybir.dt.float32
f16 = mybir.dt.float16

C = 768
H = 8
HD = 96
NG = 64  # groups
CT = C // 128  # 6 c-tiles
S_CHUNK = 256


def build_nc(b_sh=2, S=4096):
    nc = bacc.Bacc()

    query_d = nc.declare_dram_parameter("query", [b_sh, NG, C], f32, isOutput=False)
    key_d = nc.declare_dram_parameter("key_in", [b_sh, S, C], f32, isOutput=False)
    wq_d = nc.declare_dram_parameter("Wq", [C, C], f32, isOutput=False)
    wk_d = nc.declare_dram_parameter("Wk", [C, C], f32, isOutput=False)
    wv_d = nc.declare_dram_parameter("Wv", [C, C], f32, isOutput=False)
    wo_d = nc.declare_dram_parameter("Wo", [C, C], f32, isOutput=False)
    bo_d = nc.declare_dram_parameter("bo", [C], f32, isOutput=False)
    out_d = nc.declare_dram_parameter("out", [b_sh, NG, C], f32, isOutput=True)

    n_chunks = S // S_CHUNK
    n_sub = S_CHUNK // 128  # s-subtiles per chunk

    with tile.TileContext(nc) as tc:
        with (
            tc.tile_pool(name="wconst", bufs=1) as wconst,
            tc.tile_pool(name="qpool", bufs=1) as qpool,
            tc.tile_pool(name="kin", bufs=2) as kin,
            tc.tile_pool(name="keyT", bufs=2) as keyTp,
            tc.tile_pool(name="ohp", bufs=33) as ohp,
            tc.tile_pool(name="khip", bufs=2) as khip,
            tc.tile_pool(name="mxp", bufs=3) as mxp,
            tc.tile_pool(name="outp", bufs=1) as outp,
            tc.tile_pool(name="ps_a", bufs=2, space="PSUM") as ps_a,
            tc.tile_pool(name="ps_tr", bufs=2, space="PSUM") as ps_tr,
            tc.tile_pool(name="ps_g4", bufs=4, space="PSUM") as ps_g4,
        ):
            # ---- constants ----
            ident64_16 = wconst.tile([NG, NG], f16)
            make_identity(nc, ident64_16[:])
            ident64_32 = wconst.tile([NG, NG], f32)
            make_identity(nc, ident64_32[:])
            ident128_16 = wconst.tile([128, 128], f16)
            make_identity(nc, ident128_16[:])
            ident128_32 = wconst.tile([128, 128], f32)
            make_identity(nc, ident128_32[:])
            ones_row = wconst.tile([1, NG], f32)
            nc.vector.memset(ones_row[:], 1.0)
            bo_sb = wconst.tile([1, C], f32)
            nc.sync.dma_start(out=bo_sb[:], in_=bo_d[:].unsqueeze(0))

            def pe_transpose_blocks_f32(src, dst, t):
                """PE-transpose 6 f32 [128,128] blocks src[:, 128u:128u+128]
                into dst[:, u, 128t:128t+128]; 3 blocks per PSUM bank."""
                for g in range(2):
                    trp = ps_tr.tile([128, 3, 128], f32, tag="pstr")
                    for j in range(3):
                        u = 3 * g + j
                        nc.tensor.matmul(
                            trp[:, j, :],
                            src[:, 128 * u : 128 * u + 128],
                            ident128_32[:],
                            is_transpose=True,
                            start=(j == 0),
                            stop=(j == 2),
                        )
                    nc.scalar.copy(
                        out=dst[:, 3 * g : 3 * g + 3, 128 * t : 128 * t + 128],
                        in_=trp[:],
                    )

            def pe_transpose_blocks(src, dst, t, rows=128):
                """PE-transpose 6 f16 [rows,128] blocks src[:, 128u:128u+128]
                (u=0..5) into dst[:, u, 128t:128t+rows] via one batched PSUM
                bank + a single strided ACT copy."""
                ident = ident128_16 if rows == 128 else ident64_16
                trp = ps_tr.tile([128, CT, rows], f16, tag="pstr")
                for u in range(CT):
                    nc.tensor.matmul(
                        trp[:, u, :],
                        src[0:rows, 128 * u : 128 * u + 128],
                        ident[:],
                        is_transpose=True,
                        start=(u == 0),
                        stop=(u == CT - 1),
                    )
                nc.scalar.copy(out=dst[:, :, 128 * t : 128 * t + rows], in_=trp[:])

            # ---- weight prep: transpose to c-major fp16 hi/lo ----
            # wT[p, u, d] = W[d, 128u + p]
            CP = 128 * H  # d-padded width for Wq/Wk (head h at 128h..128h+96)
            wqT_h = wconst.tile([128, CT, CP], f16)
            wqT_l = wconst.tile([128, CT, CP], f16)
            # Wk kept NATURAL fp32 (d-padded rows, c free) for the
            # Y = Wk_h^T q_h precompute; logits = keyT^T @ Y in fp32
            # (~2.1 cyc/row warm measured, and exact precision)
            wk_nat = wconst.tile([128, H, C], f32)
            wvT_h = wconst.tile([128, CT, C], f16)
            woT_h = wconst.tile([128, CT, C], f16)

            # Prefetch the first key chunk so the PE has transpose work
            # while the weight tiles stream in.
            knat_pre = kin.tile([128, n_sub, C], f32, tag="knat")
            nc.gpsimd.dma_start(
                out=knat_pre[:],
                in_=key_d[0, 0:S_CHUNK, :].rearrange("(i p) c -> p i c", p=128),
            )

            # Wq/Wk use the d-padded layout (head h -> cols 128h..128h+96, rest
            # zero) so every logits matmul is a single offset-0 K=128 matmul
            # (mixed-row-offset PSUM accumulation groups fail to load on HW).
            wtmp_ctx = tc.tile_pool(name="wtmp", bufs=3)
            wtmp = wtmp_ctx.__enter__()
            for hd in range(H):
                wnat = wtmp.tile([128, C], f32, tag="wnat")
                nc.vector.memset(wnat[96:128, :], 0.0)
                nc.sync.dma_start(
                    out=wnat[0:HD, :], in_=wq_d[HD * hd : HD * hd + HD, :]
                )
                whi = wtmp.tile([128, C], f16, tag="whi")
                nc.vector.tensor_copy(whi[:], wnat[:])
                pe_transpose_blocks(whi[:], wqT_h[:], hd)
                wlo = wtmp.tile([128, C], f16, tag="wlo")
                nc.vector.tensor_tensor(
                    out=wlo[:], in0=wnat[:], in1=whi[:], op=mybir.AluOpType.subtract
                )
                pe_transpose_blocks(wlo[:], wqT_l[:], hd)
            nc.vector.memset(wk_nat[96:128, :, :], 0.0)
            for hd in range(H):
                nc.sync.dma_start(
                    out=wk_nat[0:HD, hd, :], in_=wk_d[HD * hd : HD * hd + HD, :]
                )
            for w_dram, dst_h in ((wv_d, wvT_h), (wo_d, woT_h)):
                for t in range(CT):
                    wnat = wtmp.tile([128, C], f32, tag="wnat")
                    nc.sync.dma_start(out=wnat[:], in_=w_dram[128 * t : 128 * t + 128, :])
                    whi = wtmp.tile([128, C], f16, tag="whi")
                    nc.vector.tensor_copy(whi[:], wnat[:])
                    pe_transpose_blocks(whi[:], dst_h[:], t)
            wtmp_ctx.__exit__(None, None, None)


            for b in range(b_sh):
                # ---- Q path ----
                q_nat = qpool.tile([NG, C], f32, tag="qnat")
                nc.sync.dma_start(out=q_nat[:], in_=query_d[b])
                qh_nat = qpool.tile([NG, C], f16, tag="qhnat")
                ql_nat = qpool.tile([NG, C], f16, tag="qlnat")
                nc.vector.tensor_copy(qh_nat[:], q_nat[:])
                nc.vector.tensor_tensor(
                    out=ql_nat[:], in0=q_nat[:], in1=qh_nat[:], op=mybir.AluOpType.subtract
                )
                # queryT (c-major) fp16 halves via DMA xbar ([64,128] blocks)
                qTq_h = qpool.tile([128, CT, NG], f16, tag="qTqh")
                qTq_l = qpool.tile([128, CT, NG], f16, tag="qTql")
                for qsrc, dst in ((qh_nat, qTq_h), (ql_nat, qTq_l)):
                    pe_transpose_blocks(qsrc, dst[:].unsqueeze(3).rearrange("p u n o -> p u (n o)"), 0, rows=NG)
                # q projection (natural layout, M=64), d-padded: q_pad [64, 1024]
                q_sb = qpool.tile([NG, CP], f32, tag="qsb")
                for half in range(2):
                    nsl = slice(512 * half, 512 * half + 512)
                    qp = ps_a.tile([NG, 512], f32, tag="psa")
                    first = True
                    for u in range(CT):
                        for lhsT, rhs in (
                            (qTq_h, wqT_h),
                            (qTq_h, wqT_l),
                            (qTq_l, wqT_h),
                        ):
                            nc.tensor.matmul(
                                qp[:],
                                lhsT[:, u, :],
                                rhs[:, u, nsl],
                                start=first,
                                stop=(u == CT - 1 and lhsT is qTq_l),
                            )
                            first = False
                    nc.scalar.copy(out=q_sb[:, nsl], in_=qp[:])
                # qT (padded d-major, per head) fp32 via PE transpose
                qT = qpool.tile([128, H, NG], f32, tag="qT")
                for hd in range(H):
                    trq2 = ps_a.tile([128, NG], f32, tag="psa")
                    nc.tensor.matmul(
                        trq2[:],
                        q_sb[:, 128 * hd : 128 * hd + 128],
                        ident64_32[:],
                        is_transpose=True,
                        start=True,
                        stop=True,
                    )
                    nc.scalar.copy(out=qT[:, hd, :], in_=trq2[:])
                # Y_all[c, 64h+n] = sum_d Wk[d(head h), c] * q[n, d], fp32,
                # then fp16 hi/lo split. logits = keyT^T @ Y_all (split x3).
                Y_h = qpool.tile([128, CT, 8 * NG], f16, tag="Yh")
                Y_l = qpool.tile([128, CT, 8 * NG], f16, tag="Yl")
                for u_c in range(CT):
                    yp = ps_a.tile([128, 8 * NG], f32, tag="psa")
                    csl = slice(128 * u_c, 128 * u_c + 128)
                    for hd in range(H):
                        nc.tensor.matmul(
                            yp[:, NG * hd : NG * hd + NG],
                            wk_nat[:, hd, csl],
                            qT[:, hd, :],
                            start=(hd == 0),
                            stop=(hd == H - 1),
                        )
                    nc.vector.tensor_copy(Y_h[:, u_c, :], yp[:])
                    nc.vector.tensor_tensor(
                        out=Y_l[:, u_c, :], in0=yp[:], in1=Y_h[:, u_c, :],
                        op=mybir.AluOpType.subtract,
                    )

                # ---- raw-key group-sum accumulators (head-pair packed):
                # gsr[j][n(2 heads), c-half+count] = sum_s onehot[s, n] key[s, c]
                # The v-projection is applied AFTER the 4096->64 reduction
                # (attn_h = (gs_raw_h/(cnt+1)) @ WvT_h), saving the whole
                # per-token v-projection. c is split into two passes over S
                # to fit PSUM; one-hots are retained, key re-streamed.
                gsr = [ps_g4.tile([128, 385], f32, tag="g4", name=f"gsr{_j}") for _j in range(4)]
                oh_tiles = []

                for chunk in range(n_chunks):
                    s0 = chunk * S_CHUNK
                    # load key chunk (chunk 0 of b 0 was prefetched)
                    if b == 0 and chunk == 0:
                        knat = knat_pre
                    else:
                        knat = kin.tile([128, n_sub, C], f32, tag="knat")
                        nc.gpsimd.dma_start(
                            out=knat[:],
                            in_=key_d[b, s0 : s0 + S_CHUNK, :].rearrange(
                                "(i p) c -> p i c", p=128
                            ),
                        )
                    # keyT via fp32 PE transposes, then fp16 hi/lo split
                    # (hi also feeds the v-projection)
                    keyT = keyTp.tile([128, CT, S_CHUNK], f32, tag="keyT")
                    for i in range(n_sub):
                        pe_transpose_blocks_f32(knat[:, i, :], keyT[:], i)
                    kTh = keyTp.tile([128, CT, S_CHUNK], f16, tag="kTh")
                    kTl = keyTp.tile([128, CT, S_CHUNK], f16, tag="kTl")
                    nc.vector.tensor_copy(kTh[:], keyT[:])
                    nc.vector.tensor_tensor(
                        out=kTl[:], in0=keyT[:], in1=kTh[:], op=mybir.AluOpType.subtract
                    )

                    # natural-layout f16 key, first c-half + ones column
                    khi = khip.tile([128, n_sub, 385], f16, tag="khi")
                    nc.vector.tensor_copy(khi[:, :, 0:384], knat[:, :, 0:384])
                    nc.vector.memset(khi[:, :, 384], 1.0)

                    for i in range(n_sub):
                        ssl = slice(128 * i, 128 * i + 128)
                        # logits for all 8 heads at once: lg[s, 64h+n] =
                        # sum_c keyT[c, s] Y_all[c, 64h+n], fp16 split x3.
                        # One accumulation group per PSUM bank: start only on
                        # the first matmul (zeroes the 2KB region), stop last.
                        lg = ps_a.tile([128, 8 * NG], f32, tag="psa")
                        first = True
                        for u_c in range(CT):
                            for kt, yt in ((kTh, Y_h), (kTh, Y_l), (kTl, Y_h)):
                                nc.tensor.matmul(
                                    lg[:],
                                    kt[:, u_c, ssl],
                                    yt[:, u_c, :],
                                    start=first,
                                    stop=(u_c == CT - 1 and kt is kTl),
                                )
                                first = False
                        # argmax -> one-hot via (x >= rowmax), fp16
                        mx = mxp.tile([128, H], f32, tag="mx")
                        lg3 = lg[:].rearrange("p (h n) -> p h n", h=H)
                        nc.vector.tensor_reduce(
                            out=mx[:],
                            in_=lg3,
                            axis=mybir.AxisListType.X,
                            op=mybir.AluOpType.max,
                        )
                        oh = ohp.tile([128, H * NG], f16, tag="oh")
                        nc.vector.tensor_tensor(
                            out=oh[:].rearrange("p (h n) -> p h n", h=H),
                            in0=lg3,
                            in1=mx[:].unsqueeze(2).to_broadcast((128, H, NG)),
                            op=mybir.AluOpType.is_ge,
                        )

                        oh_tiles.append(oh)
                        # pass 1: gs_raw over c[0:384] + counts (ones column)
                        last = chunk == n_chunks - 1 and i == n_sub - 1
                        first = chunk == 0 and i == 0
                        for j in range(4):
                            nc.tensor.matmul(
                                gsr[j][:],
                                oh[:, 128 * j : 128 * j + 128],
                                khi[:, i, :],
                                start=first,
                                stop=last,
                            )

                # ---- recip of counts, divide pass-1 halves into ga ----
                cnts = outp.tile([128, 4], f32, tag="cnts")
                recs = outp.tile([128, 4], f32, tag="recs")
                ga = outp.tile([128, 4, C], f16, tag="ga")
                for j in range(4):
                    nc.vector.tensor_scalar(
                        out=cnts[:, j : j + 1], in0=gsr[j][:, 384:385],
                        scalar1=1.0, scalar2=None, op0=mybir.AluOpType.add,
                    )
                    nc.vector.reciprocal(recs[:, j : j + 1], cnts[:, j : j + 1])
                    nc.vector.tensor_scalar(
                        out=ga[:, j, 0:384], in0=gsr[j][:, 0:384],
                        scalar1=recs[:, j : j + 1], scalar2=None,
                        op0=mybir.AluOpType.mult,
                    )

                # ---- pass 2: re-stream key, gs_raw over c[384:768] ----
                gsr2 = [ps_g4.tile([128, 385], f32, tag="g4", name=f"gsr2_{_j}") for _j in range(4)]
                for chunk in range(n_chunks):
                    s0 = chunk * S_CHUNK
                    knat2 = kin.tile([128, n_sub, C], f32, tag="knat")
                    nc.gpsimd.dma_start(
                        out=knat2[:],
                        in_=key_d[b, s0 : s0 + S_CHUNK, :].rearrange(
                            "(i p) c -> p i c", p=128
                        ),
                    )
                    khi2 = khip.tile([128, n_sub, 385], f16, tag="khi")
                    nc.vector.tensor_copy(khi2[:, :, 0:384], knat2[:, :, 384:768])
                    for i in range(n_sub):
                        last = chunk == n_chunks - 1 and i == n_sub - 1
                        first = chunk == 0 and i == 0
                        oh_t = oh_tiles[chunk * n_sub + i]
                        for j in range(4):
                            nc.tensor.matmul(
                                gsr2[j][:, 0:384],
                                oh_t[:, 128 * j : 128 * j + 128],
                                khi2[:, i, 0:384],
                                start=first,
                                stop=last,
                            )
                for j in range(4):
                    nc.vector.tensor_scalar(
                        out=ga[:, j, 384:768], in0=gsr2[j][:, 0:384],
                        scalar1=recs[:, j : j + 1], scalar2=None,
                        op0=mybir.AluOpType.mult,
                    )

                # ---- transpose divided gs_raw, project through WvT ----
                gaT = outp.tile([128, CT, 4, 128], f16, tag="gaT")
                for j in range(4):
                    pe_transpose_blocks(ga[:, j, :], gaT[:, :, j, :], 0)
                attn16 = outp.tile([NG, C], f16, tag="attn16")
                for h in range(H):
                    pa = ps_a.tile([NG, HD], f32, tag="psa")
                    for u_c in range(CT):
                        nc.tensor.matmul(
                            pa[:],
                            gaT[:, u_c, h // 2, 64 * (h % 2) : 64 * (h % 2) + 64],
                            wvT_h[:, u_c, HD * h : HD * h + HD],
                            start=(u_c == 0),
                            stop=(u_c == CT - 1),
                        )
                    nc.scalar.copy(out=attn16[:, HD * h : HD * h + HD], in_=pa[:])
                attnT = outp.tile([128, CT, NG], f16, tag="attnT")
                pe_transpose_blocks(attn16, attnT[:].unsqueeze(3).rearrange("p u n o -> p u (n o)"), 0, rows=NG)

                out_sb = outp.tile([NG, C], f32, tag="outsb")
                for half in range(2):
                    nsl = slice(384 * half, 384 * half + 384)
                    op = ps_a.tile([NG, 384], f32, tag="psa")
                    for u_c in range(CT):
                        nc.tensor.matmul(
                            op[:],
                            attnT[:, u_c, :],
                            woT_h[:, u_c, nsl],
                            start=(u_c == 0),
                            stop=False,
                        )
                    nc.tensor.matmul(
                        op[:], ones_row[:], bo_sb[:, nsl], start=False, stop=True
                    )
                    nc.scalar.copy(out=out_sb[:, nsl], in_=op[:])
                nc.gpsimd.dma_start(out=out_d[b], in_=out_sb[:])

    nc.finalize()
    return nc


_NC_CACHE = {}


def _get_nc(b_sh, S):
    key = (b_sh, S)
    if key not in _NC_CACHE:
        _NC_CACHE[key] = build_nc(b_sh, S)
    return _NC_CACHE[key]


def kernel(query, key_in, Wq, Wk, Wv, Wo, bo):
    from concourse.bass_utils import run_bass_kernel_spmd

    query = np.ascontiguousarray(np.asarray(query, dtype=np.float32))
    key_in = np.ascontiguousarray(np.asarray(key_in, dtype=np.float32))
    Wq = np.ascontiguousarray(np.asarray(Wq, dtype=np.float32))
    Wk = np.ascontiguousarray(np.asarray(Wk, dtype=np.float32))
    Wv = np.ascontiguousarray(np.asarray(Wv, dtype=np.float32))
    Wo = np.ascontiguousarray(np.asarray(Wo, dtype=np.float32))
    bo = np.ascontiguousarray(np.asarray(bo, dtype=np.float32))

    B, _, _ = query.shape
    S = key_in.shape[1]
    n_cores = 8
    b_sh = B // n_cores
    nc = _get_nc(b_sh, S)

    in_maps = []
    for i in range(n_cores):
        bs = slice(i * b_sh, (i + 1) * b_sh)
        in_maps.append(
            {
                "query": np.ascontiguousarray(query[bs]),
                "key_in": np.ascontiguousarray(key_in[bs]),
                "Wq": Wq,
                "Wk": Wk,
                "Wv": Wv,
                "Wo": Wo,
                "bo": bo,
            }
        )
    res = run_bass_kernel_spmd(nc, in_maps, core_ids=list(range(n_cores)))
    out = np.concatenate([res.results[i]["out"] for i in range(n_cores)], axis=0)
    return out.astype(np.float32)


if __name__ == "__main__":
    nc = build_nc(1, 512)
    print("built ok")


# revision 37
# speedup vs baseline: 1.3171x; 1.0559x over previous
"""AssignAttention (hard-routing slot attention) Trainium2 kernel, 8-core data-parallel.

Problem: B=16, N=64 groups, S=4096 tokens, C=768, H=8 heads, HD=96.
  q = query @ Wq.T; k = key @ Wk.T; v = key @ Wv.T (per-head split)
  logits = q @ k.T; hard-argmax over the 64 groups per token -> one-hot
  (softmax and the *SCALE factor are argmax-invariant, so both are skipped);
  attn = onehot / (count + 1); out = (attn @ v per head) @ Wo.T + bo

Sharding: data-parallel over batch B: 16 batches / 8 cores = 2 per core.
No collectives; the host concatenates per-core outputs.

Algorithm per core (validated vs the fp32 reference: rel_l2 ~2e-3, the
residual being argmax flips on near-ties that any reimplementation incurs):
  - The logits are REASSOCIATED: instead of projecting k = key @ Wk.T
    (the dominant 8192x768x768 matmul) and then contracting with q over
    head_dim, we precompute Y[c, (h,n)] = sum_d Wk[d(head h), c] q[n, d]
    (tiny: 768x512 per batch) and compute logits[s, (h,n)] =
    sum_c keyT[c, s] Y[c, (h,n)] -- one 768-contraction matmul produces all
    8 heads' logits, and the k-projection disappears entirely.
  - Precision on the argmax path: fp16 hi/lo split x3 matmuls
    (K@Y ~= Kh@Yh + Kh@Yl + Kl@Yh, fp32 PSUM accumulation, error ~2^-21);
    q-projection the same; Y itself fp32. Head dim is zero-padded 96->128
    because PSUM accumulation groups cannot mix matmul row offsets on HW.
  - argmax via row-max + (x >= max) compare (ties are ~1-ulp rare and only
    perturb one group's mean); counts via an extra N=1 ones-column matmul
    into the same PSUM accumulator; renorm = per-partition reciprocal.
  - keyT (c-major) via PE transpose-mode (fp32, 3 blocks batched per PSUM
    bank, single strided ACT copy out); fp16 halves derived on DVE.
  - v-projection, group-sum, Wo: single-pass fp16 (linear-path error ~5e-4);
    bias via a K=1 fp32 outer-product matmul into the same PSUM group.
  - One accumulation group per 2KB PSUM bank (start zeroes the whole bank).
  - Engine split: PE does matmuls/transposes; DVE does max/is_ge/casts;
    ACT (scalar) does PSUM->SBUF copies; SWDGE does bulk key DMA; HWDGE the
    rest. Measured ~645 us on silicon at 2.4 GHz (~1.36M PE cycles,
    TensorEngine ~88% busy; clock-state dependent).
"""

import sys

if "/opt/trn_rl_repo" not in sys.path:
    sys.path.insert(0, "/opt/trn_rl_repo")

import numpy as np

import concourse.bass as bass
import concourse.mybir as mybir
from concourse import bacc
import concourse.tile as tile
from concourse.masks import make_identity

f32 = mybir.dt.float32
f16 = mybir.dt.float16

C = 768
H = 8
HD = 96
NG = 64  # groups
CT = C // 128  # 6 c-tiles
S_CHUNK = 256


def build_nc(b_sh=2, S=4096):
    nc = bacc.Bacc()

    query_d = nc.declare_dram_parameter("query", [b_sh, NG, C], f32, isOutput=False)
    key_d = nc.declare_dram_parameter("key_in", [b_sh, S, C], f32, isOutput=False)
    wq_d = nc.declare_dram_parameter("Wq", [C, C], f32, isOutput=False)
    wk_d = nc.declare_dram_parameter("Wk", [C, C], f32, isOutput=False)
    wv_d = nc.declare_dram_parameter("Wv", [C, C], f32, isOutput=False)
    wo_d = nc.declare_dram_parameter("Wo", [C, C], f32, isOutput=False)
    bo_d = nc.declare_dram_parameter("bo", [C], f32, isOutput=False)
    out_d = nc.declare_dram_parameter("out", [b_sh, NG, C], f32, isOutput=True)

    n_chunks = S // S_CHUNK
    n_sub = S_CHUNK // 128  # s-subtiles per chunk

    with tile.TileContext(nc) as tc:
        with (
            tc.tile_pool(name="wconst", bufs=1) as wconst,
            tc.tile_pool(name="qpool", bufs=1) as qpool,
            tc.tile_pool(name="kin", bufs=2) as kin,
            tc.tile_pool(name="keyT", bufs=2) as keyTp,
            tc.tile_pool(name="ohp", bufs=33) as ohp,
            tc.tile_pool(name="khip", bufs=2) as khip,
            tc.tile_pool(name="mxp", bufs=3) as mxp,
            tc.tile_pool(name="outp", bufs=1) as outp,
            tc.tile_pool(name="ps_a", bufs=2, space="PSUM") as ps_a,
            tc.tile_pool(name="ps_tr", bufs=2, space="PSUM") as ps_tr,
            tc.tile_pool(name="ps_g4", bufs=4, space="PSUM") as ps_g4,
        ):
            # ---- constants ----
            ident64_16 = wconst.tile([NG, NG], f16)
            make_identity(nc, ident64_16[:])
            ident64_32 = wconst.tile([NG, NG], f32)
            make_identity(nc, ident64_32[:])
            ident128_16 = wconst.tile([128, 128], f16)
            make_identity(nc, ident128_16[:])
            ident128_32 = wconst.tile([128, 128], f32)
            make_identity(nc, ident128_32[:])
            ones_row = wconst.tile([1, NG], f32)
            nc.vector.memset(ones_row[:], 1.0)
            bo_sb = wconst.tile([1, C], f32)
            nc.sync.dma_start(out=bo_sb[:], in_=bo_d[:].unsqueeze(0))

            def pe_transpose_blocks_f32(src, dst, t):
                """PE-transpose 6 f32 [128,128] blocks src[:, 128u:128u+128]
                into dst[:, u, 128t:128t+128]; 3 blocks per PSUM bank."""
                for g in range(2):
                    trp = ps_tr.tile([128, 3, 128], f32, tag="pstr")
                    for j in range(3):
                        u = 3 * g + j
                        nc.tensor.matmul(
                            trp[:, j, :],
                            src[:, 128 * u : 128 * u + 128],
                            ident128_32[:],
                            is_transpose=True,
                            start=(j == 0),
                            stop=(j == 2),
                        )
                    nc.scalar.copy(
                        out=dst[:, 3 * g : 3 * g + 3, 128 * t : 128 * t + 128],
                        in_=trp[:],
                    )

            def pe_transpose_blocks(src, dst, t, rows=128):
                """PE-transpose 6 f16 [rows,128] blocks src[:, 128u:128u+128]
                (u=0..5) into dst[:, u, 128t:128t+rows] via one batched PSUM
                bank + a single strided ACT copy."""
                ident = ident128_16 if rows == 128 else ident64_16
                trp = ps_tr.tile([128, CT, rows], f16, tag="pstr")
                for u in range(CT):
                    nc.tensor.matmul(
                        trp[:, u, :],
                        src[0:rows, 128 * u : 128 * u + 128],
                        ident[:],
                        is_transpose=True,
                        start=(u == 0),
                        stop=(u == CT - 1),
                    )
                nc.scalar.copy(out=dst[:, :, 128 * t : 128 * t + rows], in_=trp[:])

            # ---- weight prep: transpose to c-major fp16 hi/lo ----
            # wT[p, u, d] = W[d, 128u + p]
            CP = 128 * H  # d-padded width for Wq/Wk (head h at 128h..128h+96)
            wqT_h = wconst.tile([128, CT, CP], f16)
            wqT_l = wconst.tile([128, CT, CP], f16)
            # Wk kept NATURAL fp32 (d-padded rows, c free) for the
            # Y = Wk_h^T q_h precompute; logits = keyT^T @ Y in fp32
            # (~2.1 cyc/row warm measured, and exact precision)
            wk_nat = wconst.tile([128, H, C], f32)
            wvT_h = wconst.tile([128, CT, C], f16)
            woT_h = wconst.tile([128, CT, C], f16)

            # Prefetch the first key chunk so the PE has transpose work
            # while the weight tiles stream in.
            knat_pre = kin.tile([128, n_sub, C], f32, tag="knat")
            nc.gpsimd.dma_start(
                out=knat_pre[:],
                in_=key_d[0, 0:S_CHUNK, :].rearrange("(i p) c -> p i c", p=128),
            )

            # Wq/Wk use the d-padded layout (head h -> cols 128h..128h+96, rest
            # zero) so every logits matmul is a single offset-0 K=128 matmul
            # (mixed-row-offset PSUM accumulation groups fail to load on HW).
            wtmp_ctx = tc.tile_pool(name="wtmp", bufs=3)
            wtmp = wtmp_ctx.__enter__()
            for hd in range(H):
                wnat = wtmp.tile([128, C], f32, tag="wnat")
                nc.vector.memset(wnat[96:128, :], 0.0)
                nc.sync.dma_start(
                    out=wnat[0:HD, :], in_=wq_d[HD * hd : HD * hd + HD, :]
                )
                whi = wtmp.tile([128, C], f16, tag="whi")
                nc.vector.tensor_copy(whi[:], wnat[:])
                pe_transpose_blocks(whi[:], wqT_h[:], hd)
                wlo = wtmp.tile([128, C], f16, tag="wlo")
                nc.vector.tensor_tensor(
                    out=wlo[:], in0=wnat[:], in1=whi[:], op=mybir.AluOpType.subtract
                )
                pe_transpose_blocks(wlo[:], wqT_l[:], hd)
            nc.vector.memset(wk_nat[96:128, :, :], 0.0)
            for hd in range(H):
                nc.sync.dma_start(
                    out=wk_nat[0:HD, hd, :], in_=wk_d[HD * hd : HD * hd + HD, :]
                )
            for w_dram, dst_h in ((wv_d, wvT_h), (wo_d, woT_h)):
                for t in range(CT):
                    wnat = wtmp.tile([128, C], f32, tag="wnat")
                    nc.sync.dma_start(out=wnat[:], in_=w_dram[128 * t : 128 * t + 128, :])
                    whi = wtmp.tile([128, C], f16, tag="whi")
                    nc.vector.tensor_copy(whi[:], wnat[:])
                    pe_transpose_blocks(whi[:], dst_h[:], t)
            wtmp_ctx.__exit__(None, None, None)


            for b in range(b_sh):
                # ---- Q path ----
                q_nat = qpool.tile([NG, C], f32, tag="qnat")
                nc.sync.dma_start(out=q_nat[:], in_=query_d[b])
                qh_nat = qpool.tile([NG, C], f16, tag="qhnat")
                ql_nat = qpool.tile([NG, C], f16, tag="qlnat")
                nc.vector.tensor_copy(qh_nat[:], q_nat[:])
                nc.vector.tensor_tensor(
                    out=ql_nat[:], in0=q_nat[:], in1=qh_nat[:], op=mybir.AluOpType.subtract
                )
                # queryT (c-major) fp16 halves via DMA xbar ([64,128] blocks)
                qTq_h = qpool.tile([128, CT, NG], f16, tag="qTqh")
                qTq_l = qpool.tile([128, CT, NG], f16, tag="qTql")
                for qsrc, dst in ((qh_nat, qTq_h), (ql_nat, qTq_l)):
                    pe_transpose_blocks(qsrc, dst[:].unsqueeze(3).rearrange("p u n o -> p u (n o)"), 0, rows=NG)
                # q projection (natural layout, M=64), d-padded: q_pad [64, 1024]
                q_sb = qpool.tile([NG, CP], f32, tag="qsb")
                for half in range(2):
                    nsl = slice(512 * half, 512 * half + 512)
                    qp = ps_a.tile([NG, 512], f32, tag="psa")
                    first = True
                    for u in range(CT):
                        for lhsT, rhs in (
                            (qTq_h, wqT_h),
                            (qTq_h, wqT_l),
                            (qTq_l, wqT_h),
                        ):
                            nc.tensor.matmul(
                                qp[:],
                                lhsT[:, u, :],
                                rhs[:, u, nsl],
                                start=first,
                                stop=(u == CT - 1 and lhsT is qTq_l),
                            )
                            first = False
                    nc.scalar.copy(out=q_sb[:, nsl], in_=qp[:])
                # qT (padded d-major, per head) fp32 via PE transpose
                qT = qpool.tile([128, H, NG], f32, tag="qT")
                for hd in range(H):
                    trq2 = ps_a.tile([128, NG], f32, tag="psa")
                    nc.tensor.matmul(
                        trq2[:],
                        q_sb[:, 128 * hd : 128 * hd + 128],
                        ident64_32[:],
                        is_transpose=True,
                        start=True,
                        stop=True,
                    )
                    nc.scalar.copy(out=qT[:, hd, :], in_=trq2[:])
                # Y_all[c, 64h+n] = sum_d Wk[d(head h), c] * q[n, d], fp32,
                # then fp16 hi/lo split. logits = keyT^T @ Y_all (split x3).
                Y_h = qpool.tile([128, CT, 8 * NG], f16, tag="Yh")
                Y_l = qpool.tile([128, CT, 8 * NG], f16, tag="Yl")
                for u_c in range(CT):
                    yp = ps_a.tile([128, 8 * NG], f32, tag="psa")
                    csl = slice(128 * u_c, 128 * u_c + 128)
                    for hd in range(H):
                        nc.tensor.matmul(
                            yp[:, NG * hd : NG * hd + NG],
                            wk_nat[:, hd, csl],
                            qT[:, hd, :],
                            start=(hd == 0),
                            stop=(hd == H - 1),
                        )
                    nc.vector.tensor_copy(Y_h[:, u_c, :], yp[:])
                    nc.vector.tensor_tensor(
                        out=Y_l[:, u_c, :], in0=yp[:], in1=Y_h[:, u_c, :],
                        op=mybir.AluOpType.subtract,
                    )

                # ---- raw-key group-sum accumulators (head-pair packed):
                # gsr[j][n(2 heads), c-half+count] = sum_s onehot[s, n] key[s, c]
                # The v-projection is applied AFTER the 4096->64 reduction
                # (attn_h = (gs_raw_h/(cnt+1)) @ WvT_h), saving the whole
                # per-token v-projection. c is split into two passes over S
                # to fit PSUM; one-hots are retained, key re-streamed.
                gsr = [ps_g4.tile([128, 385], f32, tag="g4", name=f"gsr{_j}") for _j in range(4)]
                oh_tiles = []

                for chunk in range(n_chunks):
                    s0 = chunk * S_CHUNK
                    # load key chunk (chunk 0 of b 0 was prefetched)
                    if b == 0 and chunk == 0:
                        knat = knat_pre
                    else:
                        knat = kin.tile([128, n_sub, C], f32, tag="knat")
                        nc.gpsimd.dma_start(
                            out=knat[:],
                            in_=key_d[b, s0 : s0 + S_CHUNK, :].rearrange(
                                "(i p) c -> p i c", p=128
                            ),
                        )
                    # keyT via fp32 PE transposes, then fp16 hi/lo split
                    # (hi also feeds the v-projection)
                    keyT = keyTp.tile([128, CT, S_CHUNK], f32, tag="keyT")
                    for i in range(n_sub):
                        pe_transpose_blocks_f32(knat[:, i, :], keyT[:], i)
                    kTh = keyTp.tile([128, CT, S_CHUNK], f16, tag="kTh")
                    kTl = keyTp.tile([128, CT, S_CHUNK], f16, tag="kTl")
                    nc.vector.tensor_copy(kTh[:], keyT[:])
                    nc.vector.tensor_tensor(
                        out=kTl[:], in0=keyT[:], in1=kTh[:], op=mybir.AluOpType.subtract
                    )

                    # natural-layout f16 key, first c-half + ones column
                    khi = khip.tile([128, n_sub, 385], f16, tag="khi")
                    nc.vector.tensor_copy(khi[:, :, 0:384], knat[:, :, 0:384])
                    nc.vector.memset(khi[:, :, 384], 1.0)

                    for i in range(n_sub):
                        ssl = slice(128 * i, 128 * i + 128)
                        # logits for all 8 heads at once: lg[s, 64h+n] =
                        # sum_c keyT[c, s] Y_all[c, 64h+n], fp16 split x3.
                        # One accumulation group per PSUM bank: start only on
                        # the first matmul (zeroes the 2KB region), stop last.
                        lg = ps_a.tile([128, 8 * NG], f32, tag="psa")
                        first = True
                        for u_c in range(CT):
                            for kt, yt in ((kTh, Y_h), (kTh, Y_l), (kTl, Y_h)):
                                nc.tensor.matmul(
                                    lg[:],
                                    kt[:, u_c, ssl],
                                    yt[:, u_c, :],
                                    start=first,
                                    stop=(u_c == CT - 1 and kt is kTl),
                                )
                                first = False
                        # argmax -> one-hot via (x >= rowmax), fp16
                        mx = mxp.tile([128, H], f32, tag="mx")
                        lg3 = lg[:].rearrange("p (h n) -> p h n", h=H)
                        nc.vector.tensor_reduce(
                            out=mx[:],
                            in_=lg3,
                            axis=mybir.AxisListType.X,
                            op=mybir.AluOpType.max,
                        )
                        oh = ohp.tile([128, H * NG], f16, tag="oh")
                        nc.vector.tensor_tensor(
                            out=oh[:].rearrange("p (h n) -> p h n", h=H),
                            in0=lg3,
                            in1=mx[:].unsqueeze(2).to_broadcast((128, H, NG)),
                            op=mybir.AluOpType.is_ge,
                        )

                        oh_tiles.append(oh)
                        # pass 1: gs_raw over c[0:384] + counts (ones column)
                        last = chunk == n_chunks - 1 and i == n_sub - 1
                        first = chunk == 0 and i == 0
                        for j in range(4):
                            nc.tensor.matmul(
                                gsr[j][:],
                                oh[:, 128 * j : 128 * j + 128],
                                khi[:, i, :],
                                start=first,
                                stop=last,
                            )

                # ---- recip of counts, divide pass-1 halves into ga ----
                cnts = outp.tile([128, 4], f32, tag="cnts")
                recs = outp.tile([128, 4], f32, tag="recs")
                ga = outp.tile([128, 4, C], f16, tag="ga")
                for j in range(4):
                    nc.vector.tensor_scalar(
                        out=cnts[:, j : j + 1], in0=gsr[j][:, 384:385],
                        scalar1=1.0, scalar2=None, op0=mybir.AluOpType.add,
                    )
                    nc.vector.reciprocal(recs[:, j : j + 1], cnts[:, j : j + 1])
                    nc.vector.tensor_scalar(
                        out=ga[:, j, 0:384], in0=gsr[j][:, 0:384],
                        scalar1=recs[:, j : j + 1], scalar2=None,
                        op0=mybir.AluOpType.mult,
                    )

                # ---- pass 2: re-stream key, gs_raw over c[384:768] ----
                gsr2 = [ps_g4.tile([128, 385], f32, tag="g4", name=f"gsr2_{_j}") for _j in range(4)]
                for chunk in range(n_chunks):
                    s0 = chunk * S_CHUNK
                    knat2 = kin.tile([128, n_sub, C], f32, tag="knat")
                    nc.gpsimd.dma_start(
                        out=knat2[:, :, 0:384],
                        in_=key_d[b, s0 : s0 + S_CHUNK, 384:768].rearrange(
                            "(i p) c -> p i c", p=128
                        ),
                    )
                    khi2 = khip.tile([128, n_sub, 385], f16, tag="khi")
                    nc.vector.tensor_copy(khi2[:, :, 0:384], knat2[:, :, 0:384])
                    for i in range(n_sub):
                        last = chunk == n_chunks - 1 and i == n_sub - 1
                        first = chunk == 0 and i == 0
                        oh_t = oh_tiles[chunk * n_sub + i]
                        for j in range(4):
                            nc.tensor.matmul(
                                gsr2[j][:, 0:384],
                                oh_t[:, 128 * j : 128 * j + 128],
                                khi2[:, i, 0:384],
                                start=first,
                                stop=last,
                            )
                for j in range(4):
                    nc.vector.tensor_scalar(
                        out=ga[:, j, 384:768], in0=gsr2[j][:, 0:384],
                        scalar1=recs[:, j : j + 1], scalar2=None,
                        op0=mybir.AluOpType.mult,
                    )

                # ---- transpose divided gs_raw, project through WvT ----
                gaT = outp.tile([128, CT, 4, 128], f16, tag="gaT")
                for j in range(4):
                    pe_transpose_blocks(ga[:, j, :], gaT[:, :, j, :], 0)
                attn16 = outp.tile([NG, C], f16, tag="attn16")
                for h in range(H):
                    pa = ps_a.tile([NG, HD], f32, tag="psa")
                    for u_c in range(CT):
                        nc.tensor.matmul(
                            pa[:],
                            gaT[:, u_c, h // 2, 64 * (h % 2) : 64 * (h % 2) + 64],
                            wvT_h[:, u_c, HD * h : HD * h + HD],
                            start=(u_c == 0),
                            stop=(u_c == CT - 1),
                        )
                    nc.scalar.copy(out=attn16[:, HD * h : HD * h + HD], in_=pa[:])
                attnT = outp.tile([128, CT, NG], f16, tag="attnT")
                pe_transpose_blocks(attn16, attnT[:].unsqueeze(3).rearrange("p u n o -> p u (n o)"), 0, rows=NG)

                out_sb = outp.tile([NG, C], f32, tag="outsb")
                for half in range(2):
                    nsl = slice(384 * half, 384 * half + 384)
                    op = ps_a.tile([NG, 384], f32, tag="psa")
                    for u_c in range(CT):
                        nc.tensor.matmul(
                            op[:],
                            attnT[:, u_c, :],
                            woT_h[:, u_c, nsl],
                            start=(u_c == 0),
                            stop=False,
                        )
                    nc.tensor.matmul(
                        op[:], ones_row[:], bo_sb[:, nsl], start=False, stop=True
                    )
                    nc.scalar.copy(out=out_sb[:, nsl], in_=op[:])
                nc.gpsimd.dma_start(out=out_d[b], in_=out_sb[:])

    nc.finalize()
    return nc


_NC_CACHE = {}


def _get_nc(b_sh, S):
    key = (b_sh, S)
    if key not in _NC_CACHE:
        _NC_CACHE[key] = build_nc(b_sh, S)
    return _NC_CACHE[key]


def kernel(query, key_in, Wq, Wk, Wv, Wo, bo):
    from concourse.bass_utils import run_bass_kernel_spmd

    query = np.ascontiguousarray(np.asarray(query, dtype=np.float32))
    key_in = np.ascontiguousarray(np.asarray(key_in, dtype=np.float32))
    Wq = np.ascontiguousarray(np.asarray(Wq, dtype=np.float32))
    Wk = np.ascontiguousarray(np.asarray(Wk, dtype=np.float32))
    Wv = np.ascontiguousarray(np.asarray(Wv, dtype=np.float32))
    Wo = np.ascontiguousarray(np.asarray(Wo, dtype=np.float32))
    bo = np.ascontiguousarray(np.asarray(bo, dtype=np.float32))

    B, _, _ = query.shape
    S = key_in.shape[1]
    n_cores = 8
    b_sh = B // n_cores
    nc = _get_nc(b_sh, S)

    in_maps = []
    for i in range(n_cores):
        bs = slice(i * b_sh, (i + 1) * b_sh)
        in_maps.append(
            {
                "query": np.ascontiguousarray(query[bs]),
                "key_in": np.ascontiguousarray(key_in[bs]),
                "Wq": Wq,
                "Wk": Wk,
                "Wv": Wv,
                "Wo": Wo,
                "bo": bo,
            }
        )
    res = run_bass_kernel_spmd(nc, in_maps, core_ids=list(range(n_cores)))
    out = np.concatenate([res.results[i]["out"] for i in range(n_cores)], axis=0)
    return out.astype(np.float32)


if __name__ == "__main__":
    nc = build_nc(1, 512)
    print("built ok")
